# revision 14
# baseline (speedup 1.0000x reference)
"""Trainium2 Bass kernel for EnhancedCrossAttention (dense transformer, 8-core SPMD).

Sharding: cores 0-3 compute gene_out rows [1024*i, 1024*(i+1)) attending over all
drug K/V; cores 4-7 mirror for drug_out. One SPMD program with direction-generic
input names; host slices/replicates inputs and concatenates outputs.

Layout strategy: all activations live transposed [dims, seq] on-chip (loaded via
DMA-xbar transpose); LayerNorm is folded into the projection matmuls as rank-1
PSUM corrections, so q/k/v projections consume the raw transposed embeds
directly and produce qT/kT ready for the score matmuls with no PE transposes.
Scores are computed transposed [k, q] so the exp'd tile is directly the lhsT of
the context matmul; a ones-column in V yields softmax denominators for free.
"""
import numpy as np
import ml_dtypes

import concourse.bass as bass
import concourse.mybir as mybir
import concourse.tile as tile
from concourse import bacc
from concourse.bass_utils import run_bass_kernel_spmd

F32 = mybir.dt.float32
BF16 = mybir.dt.bfloat16
AF = mybir.ActivationFunctionType
ALU = mybir.AluOpType

D = 512
H = 8
DH = 64
S_OWN = 1024   # query rows per core
S_OTH = 4096   # key/value rows (full opposite side)
NC = 8

LN_EPS = 1e-5
L2_EPS2 = 1e-24          # eps^2 for l2 norm (ref: max(norm, 1e-12))
QSCALE_LOG = float(np.log(0.125))  # DH ** -0.5 folded into inv-norm of q


def build_nc():
    nc = bacc.Bacc("TRN2", target_bir_lowering=False, debug=False, num_devices=NC)

    # ---- DRAM I/O ----
    x16_own = nc.dram_tensor("x16_own", [S_OWN, D], BF16, kind="ExternalInput")
    x16_oth = nc.dram_tensor("x16_oth", [S_OTH, D], BF16, kind="ExternalInput")
    xf_own = nc.dram_tensor("xf_own", [S_OWN, D], F32, kind="ExternalInput")
    wq_d = nc.dram_tensor("wq", [D, D], F32, kind="ExternalInput")
    wk_d = nc.dram_tensor("wk", [D, D], F32, kind="ExternalInput")
    wv_d = nc.dram_tensor("wv", [D, D], F32, kind="ExternalInput")
    wo_d = nc.dram_tensor("wo", [D, D], F32, kind="ExternalInput")
    wg_d = nc.dram_tensor("wg", [2 * D, D], F32, kind="ExternalInput")
    bq_d = nc.dram_tensor("bq", [D], F32, kind="ExternalInput")
    bk_d = nc.dram_tensor("bk", [D], F32, kind="ExternalInput")
    bv_d = nc.dram_tensor("bv", [D], F32, kind="ExternalInput")
    bo_d = nc.dram_tensor("bo", [D], F32, kind="ExternalInput")
    bg_d = nc.dram_tensor("bg", [D], F32, kind="ExternalInput")
    g_own_d = nc.dram_tensor("g_own", [D], F32, kind="ExternalInput")
    b_own_d = nc.dram_tensor("b_own", [D], F32, kind="ExternalInput")
    g_oth_d = nc.dram_tensor("g_oth", [D], F32, kind="ExternalInput")
    b_oth_d = nc.dram_tensor("b_oth", [D], F32, kind="ExternalInput")
    gg_d = nc.dram_tensor("gg", [D], F32, kind="ExternalInput")
    gb_d = nc.dram_tensor("gb", [D], F32, kind="ExternalInput")
    out_d = nc.dram_tensor("out", [S_OWN, D], F32, kind="ExternalOutput")

    # DRAM scratch for row replication round-trips
    scr_stats = {}
    for side, s in (("own", S_OWN), ("oth", S_OTH)):
        scr_stats[side] = {
            "mu": nc.dram_tensor(f"scr_mu_{side}", [s], F32),
            "m2": nc.dram_tensor(f"scr_m2_{side}", [s], F32),
            "negmu": nc.dram_tensor(f"scr_negmu_{side}", [s], BF16),
            "rstd": nc.dram_tensor(f"scr_rstd_{side}", [s], BF16),
            "invr": nc.dram_tensor(f"scr_invr_{side}", [s], BF16),
        }
    scr_ssq_q = nc.dram_tensor("scr_ssq_q", [H, S_OWN], F32)
    scr_ssq_k = nc.dram_tensor("scr_ssq_k", [H, S_OTH], F32)
    scr_inv_q = nc.dram_tensor("scr_inv_q", [H, S_OWN], BF16)
    scr_inv_k = nc.dram_tensor("scr_inv_k", [H, S_OTH], BF16)
    scr_rsum = nc.dram_tensor("scr_rsum", [H, S_OWN], BF16)

    def bcast_ap(dram, offset, nrep, n):
        return bass.AP(tensor=dram, offset=offset, ap=[[0, nrep], [1, n]])

    with tile.TileContext(nc) as tc:
        with tc.tile_pool(name="persist", bufs=1) as persist:
            # ---- constants ----
            ones_col = persist.tile([128, 1], BF16)       # K=128->M=1 colsum
            nc.vector.memset(ones_col, 1.0)
            oD_col = persist.tile([128, 1], BF16)         # 1/D for mean
            nc.vector.memset(oD_col, 1.0 / D)
            ones_row = persist.tile([1, 128], BF16)       # K=1 lhsT for bias outer
            nc.vector.memset(ones_row, 1.0)
            hsel = persist.tile([128, 2], BF16)           # per-head-pair colsum
            nc.vector.memset(hsel, 0.0)
            nc.vector.memset(hsel[0:64, 0:1], 1.0)
            nc.vector.memset(hsel[64:128, 1:2], 1.0)
            eps_col = persist.tile([128, 1], F32)
            nc.vector.memset(eps_col, LN_EPS)
            eps24_col = persist.tile([128, 1], F32)
            nc.vector.memset(eps24_col, L2_EPS2)
            qlog_col = persist.tile([128, 1], F32)
            nc.vector.memset(qlog_col, QSCALE_LOG)
            zero_col = persist.tile([128, 1], F32)
            nc.vector.memset(zero_col, 0.0)

            # ---- persistent SBUF tensors ----
            xT_own = persist.tile([128, 4, S_OWN], BF16)
            qT = persist.tile([128, 4, S_OWN], BF16)
            kT = persist.tile([128, 4, S_OTH], BF16)
            vsb = persist.tile([128, 32, H, DH + 1], BF16)
            ctxT = persist.tile([128, 4, S_OWN], BF16)
            wqb = persist.tile([128, 4, D], BF16)
            wkb = persist.tile([128, 4, D], BF16)
            wvb = persist.tile([128, 4, D], BF16)
            wob = persist.tile([128, 4, D], BF16)
            wgb = persist.tile([128, 8, D], BF16)
            csum_q = persist.tile([1, D], BF16)
            csum_k = persist.tile([1, D], BF16)
            bp_q = persist.tile([1, D], BF16)
            bp_k = persist.tile([1, D], BF16)
            bp_v = persist.tile([1, D], BF16)
            bo_row = persist.tile([1, D], BF16)
            bg_row = persist.tile([1, D], BF16)
            gg_rep = persist.tile([128, D], F32)
            gb_rep = persist.tile([128, D], F32)
            negmu = {"own": persist.tile([1, S_OWN], BF16, name="negmu_own"),
                     "oth": persist.tile([1, S_OTH], BF16, name="negmu_oth")}
            invr = {"own": persist.tile([1, S_OWN], BF16, name="invr_own"),
                    "oth": persist.tile([1, S_OTH], BF16, name="invr_oth")}
            rstd_rep_own = persist.tile([128, S_OWN], BF16)

            # ================= P1: weight prep =================
            with tc.tile_pool(name="wstage", bufs=2) as wstage, \
                 tc.tile_pool(name="wpsum", bufs=2, space="PSUM") as wpsum:
                gcols = {}
                for nm, dram in (("g_own", g_own_d), ("b_own", b_own_d),
                                 ("g_oth", g_oth_d), ("b_oth", b_oth_d)):
                    t = wstage.tile([128, 4], F32, tag=f"gcol_{nm}")
                    nc.sync.dma_start(out=t[:, :],
                                      in_=dram.ap().rearrange("(c p) -> p c", p=128))
                    gcols[nm] = t

                def prep_qk(w_dram, b_dram, g_nm, bln_nm, wb, csum, bp):
                    wst = wstage.tile([128, 4, D], F32, tag="wst")
                    nc.sync.dma_start(out=wst[:, :, :],
                                      in_=w_dram.ap().rearrange("(c p) d -> p c d", p=128))
                    for c in range(4):
                        nc.vector.tensor_scalar_mul(out=wb[:, c, :], in0=wst[:, c, :],
                                                    scalar1=gcols[g_nm][:, c:c + 1])
                    ps = wpsum.tile([1, D], F32, tag="wps")
                    for c in range(4):
                        nc.tensor.matmul(ps[:, :], ones_col[:, :], wb[:, c, :],
                                         start=(c == 0), stop=(c == 3))
                    nc.vector.tensor_copy(out=csum[:, :], in_=ps[:, :])
                    ps2 = wpsum.tile([1, D], F32, tag="wps")
                    for c in range(4):
                        nc.tensor.matmul(ps2[:, :], gcols[bln_nm][:, c:c + 1],
                                         wst[:, c, :], start=(c == 0), stop=(c == 3))
                    brow = wstage.tile([1, D], F32, tag="brow")
                    nc.sync.dma_start(out=brow[:, :], in_=b_dram.ap()[None, :])
                    bsum = wstage.tile([1, D], F32, tag="bsum")
                    nc.vector.tensor_add(out=bsum[:, :], in0=ps2[:, :], in1=brow[:, :])
                    nc.vector.tensor_copy(out=bp[:, :], in_=bsum[:, :])

                prep_qk(wq_d, bq_d, "g_own", "b_own", wqb, csum_q, bp_q)
                prep_qk(wk_d, bk_d, "g_oth", "b_oth", wkb, csum_k, bp_k)

                for w_dram, wb in ((wv_d, wvb), (wo_d, wob)):
                    wst = wstage.tile([128, 4, D], F32, tag="wst")
                    nc.sync.dma_start(out=wst[:, :, :],
                                      in_=w_dram.ap().rearrange("(c p) d -> p c d", p=128))
                    for c in range(4):
                        nc.vector.tensor_copy(out=wb[:, c, :], in_=wst[:, c, :])
                wst8 = wstage.tile([128, 8, D], F32, tag="wst8")
                nc.sync.dma_start(out=wst8[:, :, :],
                                  in_=wg_d.ap().rearrange("(c p) d -> p c d", p=128))
                for c in range(8):
                    nc.vector.tensor_copy(out=wgb[:, c, :], in_=wst8[:, c, :])

                for b_dram, row in ((bv_d, bp_v), (bo_d, bo_row), (bg_d, bg_row)):
                    br = wstage.tile([1, D], F32, tag="brow")
                    nc.sync.dma_start(out=br[:, :], in_=b_dram.ap()[None, :])
                    nc.vector.tensor_copy(out=row[:, :], in_=br[:, :])

                for dram, rep in ((gg_d, gg_rep), (gb_d, gb_rep)):
                    nc.sync.dma_start(out=rep[:, :], in_=bcast_ap(dram, 0, 128, D))

            # ================= P2-P5 (need xT_oth alive) =================
            xT_oth_cm = tc.tile_pool(name="xT_oth", bufs=1)
            xT_oth_pool = xT_oth_cm.__enter__()
            xT_oth = xT_oth_pool.tile([128, 4, S_OTH], BF16)
            for c in range(4):
                nc.sync.dma_start_transpose(out=xT_own[:, c, :],
                                            in_=x16_own.ap()[:, c * 128:(c + 1) * 128])
                nc.sync.dma_start_transpose(out=xT_oth[:, c, :],
                                            in_=x16_oth.ap()[:, c * 128:(c + 1) * 128])

            # ---- P3: LN stats ----
            with tc.tile_pool(name="sq", bufs=2) as sqp, \
                 tc.tile_pool(name="stps", bufs=2, space="PSUM") as stps:
                for side, s, xt in (("own", S_OWN, xT_own), ("oth", S_OTH, xT_oth)):
                    nwin = s // 512
                    for w in range(nwin):
                        wsl = slice(w * 512, (w + 1) * 512)
                        ps_mu = stps.tile([1, 512], F32, tag="psmu")
                        ps_m2 = stps.tile([1, 512], F32, tag="psm2")
                        for c in range(4):
                            sq = sqp.tile([128, 512], BF16, tag="sq")
                            nc.vector.tensor_mul(out=sq[:, :], in0=xt[:, c, wsl],
                                                 in1=xt[:, c, wsl])
                            nc.tensor.matmul(ps_mu[:, :], oD_col[:, :], xt[:, c, wsl],
                                             start=(c == 0), stop=(c == 3))
                            nc.tensor.matmul(ps_m2[:, :], oD_col[:, :], sq[:, :],
                                             start=(c == 0), stop=(c == 3))
                        strow_mu = sqp.tile([1, 512], F32, tag="strow_mu")
                        strow_m2 = sqp.tile([1, 512], F32, tag="strow_m2")
                        nc.vector.tensor_copy(out=strow_mu[:, :], in_=ps_mu[:, :])
                        nc.vector.tensor_copy(out=strow_m2[:, :], in_=ps_m2[:, :])
                        nc.sync.dma_start(
                            out=scr_stats[side]["mu"].ap()[wsl][None, :],
                            in_=strow_mu[:, :])
                        nc.sync.dma_start(
                            out=scr_stats[side]["m2"].ap()[wsl][None, :],
                            in_=strow_m2[:, :])
                for side, s in (("own", S_OWN), ("oth", S_OTH)):
                    fcol = s // 128
                    mu_pk = sqp.tile([128, fcol], F32, tag="pk_mu")
                    m2_pk = sqp.tile([128, fcol], F32, tag="pk_m2")
                    nc.sync.dma_start(
                        out=mu_pk[:, :],
                        in_=scr_stats[side]["mu"].ap().rearrange("(p f) -> p f", p=128))
                    nc.sync.dma_start(
                        out=m2_pk[:, :],
                        in_=scr_stats[side]["m2"].ap().rearrange("(p f) -> p f", p=128))
                    msq = sqp.tile([128, fcol], F32, tag="pk_msq")
                    nc.scalar.activation(out=msq[:, :], in_=mu_pk[:, :], func=AF.Square)
                    var = sqp.tile([128, fcol], F32, tag="pk_var")
                    nc.vector.tensor_sub(out=var[:, :], in0=m2_pk[:, :], in1=msq[:, :])
                    lnv = sqp.tile([128, fcol], F32, tag="pk_lnv")
                    nc.scalar.activation(out=lnv[:, :], in_=var[:, :], func=AF.Ln,
                                         bias=eps_col[:, :])
                    rstd_pk = sqp.tile([128, fcol], BF16, tag="pk_rstd")
                    nc.scalar.activation(out=rstd_pk[:, :], in_=lnv[:, :], func=AF.Exp,
                                         scale=-0.5)
                    invr_pk = sqp.tile([128, fcol], BF16, tag="pk_invr")
                    nc.scalar.activation(out=invr_pk[:, :], in_=lnv[:, :], func=AF.Exp,
                                         scale=0.5)
                    nmu_pk = sqp.tile([128, fcol], BF16, tag="pk_nmu")
                    nc.vector.tensor_scalar_mul(out=nmu_pk[:, :], in0=mu_pk[:, :],
                                                scalar1=-1.0)
                    for nm, pk in (("negmu", nmu_pk), ("rstd", rstd_pk),
                                   ("invr", invr_pk)):
                        nc.sync.dma_start(
                            out=scr_stats[side][nm].ap().rearrange("(p f) -> p f", p=128),
                            in_=pk[:, :])
                for side in ("own", "oth"):
                    nc.sync.dma_start(out=negmu[side][:, :],
                                      in_=scr_stats[side]["negmu"].ap()[None, :])
                    nc.sync.dma_start(out=invr[side][:, :],
                                      in_=scr_stats[side]["invr"].ap()[None, :])
                nc.sync.dma_start(out=rstd_rep_own[:, :],
                                  in_=bcast_ap(scr_stats["own"]["rstd"], 0, 128, S_OWN))

            # ---- P4: projections ----
            with tc.tile_pool(name="prps", bufs=2, space="PSUM") as prps, \
                 tc.tile_pool(name="prtmp", bufs=1) as prtmp:
                rstd_rep_oth = prtmp.tile([128, S_OTH], BF16, tag="rstdoth")
                nc.sync.dma_start(out=rstd_rep_oth[:, :],
                                  in_=bcast_ap(scr_stats["oth"]["rstd"], 0, 128, S_OTH))
                for oc in range(4):
                    osl = slice(oc * 128, (oc + 1) * 128)
                    for w in range(S_OWN // 512):
                        wsl = slice(w * 512, (w + 1) * 512)
                        ps = prps.tile([128, 512], F32, tag="pps")
                        for c in range(4):
                            nc.tensor.matmul(ps[:, :], wqb[:, c, osl],
                                             xT_own[:, c, wsl],
                                             start=(c == 0), stop=False)
                        nc.tensor.matmul(ps[:, :], csum_q[:, osl],
                                         negmu["own"][:, wsl], start=False, stop=False)
                        nc.tensor.matmul(ps[:, :], bp_q[:, osl],
                                         invr["own"][:, wsl], start=False, stop=True)
                        nc.vector.tensor_mul(out=qT[:, oc, wsl], in0=ps[:, :],
                                             in1=rstd_rep_own[:, wsl])
                for oc in range(4):
                    osl = slice(oc * 128, (oc + 1) * 128)
                    for w in range(S_OTH // 512):
                        wsl = slice(w * 512, (w + 1) * 512)
                        ps = prps.tile([128, 512], F32, tag="pps")
                        for c in range(4):
                            nc.tensor.matmul(ps[:, :], wkb[:, c, osl],
                                             xT_oth[:, c, wsl],
                                             start=(c == 0), stop=False)
                        nc.tensor.matmul(ps[:, :], csum_k[:, osl],
                                         negmu["oth"][:, wsl], start=False, stop=False)
                        nc.tensor.matmul(ps[:, :], bp_k[:, osl],
                                         invr["oth"][:, wsl], start=False, stop=True)
                        nc.vector.tensor_mul(out=kT[:, oc, wsl], in0=ps[:, :],
                                             in1=rstd_rep_oth[:, wsl])
                for sb in range(32):
                    ps = prps.tile([128, D], F32, tag="vps")
                    ssl = slice(sb * 128, (sb + 1) * 128)
                    for c in range(4):
                        nc.tensor.matmul(ps[:, :], xT_oth[:, c, ssl], wvb[:, c, :],
                                         start=(c == 0), stop=False)
                    nc.tensor.matmul(ps[:, :], ones_row[:, :], bp_v[:, :],
                                     start=False, stop=True)
                    nc.vector.tensor_copy(
                        out=vsb[:, sb, :, 0:DH],
                        in_=ps[:, :].rearrange("p (h d) -> p h d", h=H))
                nc.vector.memset(vsb[:, :, :, DH:DH + 1], 1.0)

            xT_oth_cm.__exit__(None, None, None)

            # ---- P5: l2 normalization of q/k ----
            with tc.tile_pool(name="l2", bufs=2) as l2p, \
                 tc.tile_pool(name="l2ps", bufs=2, space="PSUM") as l2ps:
                for (name, t, s, scr_ssq, scr_inv) in (
                        ("q", qT, S_OWN, scr_ssq_q, scr_inv_q),
                        ("k", kT, S_OTH, scr_ssq_k, scr_inv_k)):
                    nwin = s // 512
                    for oc in range(4):
                        for w in range(nwin):
                            wsl = slice(w * 512, (w + 1) * 512)
                            sq = l2p.tile([128, 512], BF16, tag="l2sq")
                            nc.vector.tensor_mul(out=sq[:, :], in0=t[:, oc, wsl],
                                                 in1=t[:, oc, wsl])
                            ps = l2ps.tile([2, 512], F32, tag="l2ps")
                            nc.tensor.matmul(ps[:, :], hsel[:, :], sq[:, :],
                                             start=True, stop=True)
                            ssrow = l2p.tile([2, 512], F32, tag="ssrow")
                            nc.vector.tensor_copy(out=ssrow[:, :], in_=ps[:, :])
                            nc.sync.dma_start(out=scr_ssq.ap()[2 * oc:2 * oc + 1, wsl],
                                              in_=ssrow[0:1, :])
                            nc.sync.dma_start(
                                out=scr_ssq.ap()[2 * oc + 1:2 * oc + 2, wsl],
                                in_=ssrow[1:2, :])
                    fcol = H * s // 128
                    pk = l2p.tile([128, fcol], F32, tag=f"l2pk_{name}")
                    nc.sync.dma_start(
                        out=pk[:, :].rearrange("p (h f) -> p h f", h=H),
                        in_=scr_ssq.ap().rearrange("h (p f) -> p h f", p=128))
                    lns = l2p.tile([128, fcol], F32, tag=f"l2ln_{name}")
                    nc.scalar.activation(out=lns[:, :], in_=pk[:, :], func=AF.Ln,
                                         bias=eps24_col[:, :])
                    ipk = l2p.tile([128, fcol], BF16, tag=f"l2in_{name}")
                    nc.scalar.activation(out=ipk[:, :], in_=lns[:, :], func=AF.Exp,
                                         scale=-0.5,
                                         bias=(qlog_col[:, :] if name == "q" else zero_col[:, :]))
                    nc.sync.dma_start(
                        out=scr_inv.ap().rearrange("h (p f) -> p h f", p=128),
                        in_=ipk[:, :].rearrange("p (h f) -> p h f", h=H))
                    for oc in range(4):
                        rep = l2p.tile([128, S_OTH], BF16, name="l2rep", tag="l2rep")[:, :s]
                        nc.sync.dma_start(out=rep[0:64, :],
                                          in_=bcast_ap(scr_inv, (2 * oc) * s, 64, s))
                        nc.sync.dma_start(out=rep[64:128, :],
                                          in_=bcast_ap(scr_inv, (2 * oc + 1) * s, 64, s))
                        nc.vector.tensor_mul(out=t[:, oc, :], in0=t[:, oc, :],
                                             in1=rep[:, :])

            # ================= P6: attention =================
            with tc.tile_pool(name="scps", bufs=2, space="PSUM") as scps, \
                 tc.tile_pool(name="ctps", bufs=2, space="PSUM") as ctps, \
                 tc.tile_pool(name="att", bufs=4) as attp, \
                 tc.tile_pool(name="attr", bufs=2) as attrp:
                for qw in range(2):
                    qsl = slice(qw * 512, (qw + 1) * 512)
                    for hp in range(4):
                        ctx2 = [ctps.tile([DH + 1, 512], F32, name=f"ctx{j}",
                                          tag=f"ctx{j}")
                                for j in range(2)]
                        for kc in range(32):
                            ksl = slice(kc * 128, (kc + 1) * 128)
                            for j in range(2):
                                psl = slice(64 * j, 64 * (j + 1))
                                sc = scps.tile([128, 512], F32, tag=f"sc{j}")
                                nc.tensor.matmul(sc[:, :], kT[psl, hp, ksl],
                                                 qT[psl, hp, qsl],
                                                 start=True, stop=True)
                                e = attp.tile([128, 512], BF16, tag=f"e{j}")
                                nc.scalar.activation(out=e[:, :], in_=sc[:, :],
                                                     func=AF.Exp)
                                nc.tensor.matmul(ctx2[j][:, :],
                                                 vsb[:, kc, 2 * hp + j, :], e[:, :],
                                                 start=(kc == 0), stop=(kc == 31))
                        for j in range(2):
                            h = 2 * hp + j
                            rs = attrp.tile([DH + 1, 512], F32, tag="rs")
                            nc.vector.reciprocal(out=rs[DH:DH + 1, :],
                                                 in_=ctx2[j][DH:DH + 1, :])
                            rs16 = attrp.tile([DH + 1, 512], BF16, tag="rs16")
                            nc.vector.tensor_copy(out=rs16[DH:DH + 1, :],
                                                  in_=rs[DH:DH + 1, :])
                            nc.sync.dma_start(out=scr_rsum.ap()[h:h + 1, qsl],
                                              in_=rs16[DH:DH + 1, :])
                            rep = attrp.tile([64, 512], BF16, tag="rsrep")
                            nc.sync.dma_start(
                                out=rep[:, :],
                                in_=bcast_ap(scr_rsum, h * S_OWN + qw * 512, 64, 512))
                            nc.vector.tensor_mul(
                                out=ctxT[64 * j:64 * (j + 1), hp, qsl],
                                in0=ctx2[j][0:DH, :], in1=rep[:, :])

            # ================= P7: output proj + gate + residual =================
            with tc.tile_pool(name="ops", bufs=2, space="PSUM") as opsp, \
                 tc.tile_pool(name="fin", bufs=3) as finp:
                for sb in range(8):
                    ssl = slice(sb * 128, (sb + 1) * 128)
                    ps_o = opsp.tile([128, D], F32, tag="pso")
                    for c in range(4):
                        nc.tensor.matmul(ps_o[:, :], ctxT[:, c, ssl], wob[:, c, :],
                                         start=(c == 0), stop=False)
                    nc.tensor.matmul(ps_o[:, :], ones_row[:, :], bo_row[:, :],
                                     start=False, stop=True)
                    proj = finp.tile([128, D], F32, tag="proj")
                    nc.vector.tensor_copy(out=proj[:, :], in_=ps_o[:, :])

                    ps_z = opsp.tile([128, D], F32, tag="psz")
                    for c in range(4):
                        nc.tensor.matmul(ps_z[:, :], ctxT[:, c, ssl], wgb[:, c, :],
                                         start=(c == 0), stop=False)
                    for c in range(4):
                        nc.tensor.matmul(ps_z[:, :], xT_own[:, c, ssl],
                                         wgb[:, 4 + c, :], start=False, stop=False)
                    nc.tensor.matmul(ps_z[:, :], ones_row[:, :], bg_row[:, :],
                                     start=False, stop=True)
                    z = finp.tile([128, D], F32, tag="z")
                    nc.vector.tensor_copy(out=z[:, :], in_=ps_z[:, :])
                    stats = finp.tile([128, 6], F32, tag="st6")
                    nc.vector.bn_stats(out=stats[:, :], in_=z[:, :])
                    mv = finp.tile([128, 2], F32, tag="mv")
                    nc.vector.bn_aggr(out=mv[:, :], in_=stats[:, :])
                    lnv = finp.tile([128, 1], F32, tag="lnv")
                    nc.scalar.activation(out=lnv[:, :], in_=mv[:, 1:2], func=AF.Ln,
                                         bias=eps_col[:, :])
                    rstd = finp.tile([128, 1], F32, tag="rstd")
                    nc.scalar.activation(out=rstd[:, :], in_=lnv[:, :], func=AF.Exp,
                                         scale=-0.5)
                    zn = finp.tile([128, D], F32, tag="zn")
                    nc.vector.tensor_scalar(out=zn[:, :], in0=z[:, :],
                                            scalar1=mv[:, 0:1], scalar2=rstd[:, :],
                                            op0=ALU.subtract, op1=ALU.mult)
                    zg = finp.tile([128, D], F32, tag="zg")
                    nc.vector.tensor_mul(out=zg[:, :], in0=zn[:, :], in1=gg_rep[:, :])
                    nc.vector.tensor_add(out=zg[:, :], in0=zg[:, :], in1=gb_rep[:, :])
                    ex = finp.tile([128, D], F32, tag="ex")
                    nc.scalar.activation(out=ex[:, :], in_=zg[:, :], func=AF.Exp,
                                         scale=-1.0)
                    nc.vector.tensor_scalar_add(out=ex[:, :], in0=ex[:, :], scalar1=1.0)
                    gate = finp.tile([128, D], F32, tag="gate")
                    nc.vector.reciprocal(out=gate[:, :], in_=ex[:, :])

                    xblk = finp.tile([128, D], F32, tag="xblk")
                    nc.sync.dma_start(out=xblk[:, :], in_=xf_own.ap()[ssl, :])
                    gp = finp.tile([128, D], F32, tag="gp")
                    nc.vector.tensor_mul(out=gp[:, :], in0=gate[:, :], in1=proj[:, :])
                    ob = finp.tile([128, D], F32, tag="ob")
                    nc.vector.tensor_add(out=ob[:, :], in0=gp[:, :], in1=xblk[:, :])
                    nc.sync.dma_start(out=out_d.ap()[ssl, :], in_=ob[:, :])

    nc.compile()
    return nc


_NC_CACHE = None


def _get_nc():
    global _NC_CACHE
    if _NC_CACHE is None:
        _NC_CACHE = build_nc()
    return _NC_CACHE


def kernel(**inputs):
    nc = _get_nc()
    xg = np.ascontiguousarray(np.asarray(inputs["gene_embeds"], np.float32))
    xd = np.ascontiguousarray(np.asarray(inputs["drug_embeds"], np.float32))
    xg16 = xg.astype(ml_dtypes.bfloat16)
    xd16 = xd.astype(ml_dtypes.bfloat16)

    f32 = lambda k: np.ascontiguousarray(np.asarray(inputs[k], np.float32))

    gene_common = dict(
        x16_oth=xd16, wq=f32("wgq"), wk=f32("wdk"), wv=f32("wdv"), wo=f32("wo"),
        wg=f32("wgg"), bq=f32("bgq"), bk=f32("bdk"), bv=f32("bdv"), bo=f32("bo"),
        bg=f32("bgg"), g_own=f32("lng_g"), b_own=f32("lng_b"), g_oth=f32("lnd_g"),
        b_oth=f32("lnd_b"), gg=f32("gg_g"), gb=f32("gg_b"))
    drug_common = dict(
        x16_oth=xg16, wq=f32("wdq"), wk=f32("wgk"), wv=f32("wgv"), wo=f32("wo"),
        wg=f32("wdg"), bq=f32("bdq"), bk=f32("bgk"), bv=f32("bgv"), bo=f32("bo"),
        bg=f32("bdg"), g_own=f32("lnd_g"), b_own=f32("lnd_b"), g_oth=f32("lng_g"),
        b_oth=f32("lng_b"), gg=f32("dg_g"), gb=f32("dg_b"))

    in_maps = []
    for i in range(8):
        if i < 4:
            sl = slice(i * S_OWN, (i + 1) * S_OWN)
            m = dict(gene_common)
            m["x16_own"] = np.ascontiguousarray(xg16[sl])
            m["xf_own"] = np.ascontiguousarray(xg[sl])
        else:
            sl = slice((i - 4) * S_OWN, (i - 3) * S_OWN)
            m = dict(drug_common)
            m["x16_own"] = np.ascontiguousarray(xd16[sl])
            m["xf_own"] = np.ascontiguousarray(xd[sl])
        in_maps.append(m)

    res = run_bass_kernel_spmd(nc, in_maps, core_ids=list(range(8)))
    gene_out = np.concatenate([res.results[i]["out"] for i in range(4)], axis=0)
    drug_out = np.concatenate([res.results[i]["out"] for i in range(4, 8)], axis=0)
    return (gene_out, drug_out)


# revision 15
# speedup vs baseline: 3708.9751x; 3708.9751x over previous
"""Trainium2 Bass kernel for EnhancedCrossAttention (dense transformer, 8-core SPMD).

Sharding: cores 0-3 compute gene_out rows [1024*i, 1024*(i+1)) attending over all
drug K/V; cores 4-7 mirror for drug_out. One SPMD program with direction-generic
input names; host slices/replicates inputs and concatenates outputs.

Layout strategy: all activations live transposed [dims, seq] on-chip (loaded via
DMA-xbar transpose); LayerNorm is folded into the projection matmuls as rank-1
PSUM corrections, so q/k/v projections consume the raw transposed embeds
directly and produce qT/kT ready for the score matmuls with no PE transposes.
Scores are computed transposed [k, q] so the exp'd tile is directly the lhsT of
the context matmul; a ones-column in V yields softmax denominators for free.
"""
import numpy as np
import ml_dtypes

import concourse.bass as bass
import concourse.mybir as mybir
import concourse.tile as tile
from concourse import bacc
from concourse.bass_utils import run_bass_kernel_spmd

F32 = mybir.dt.float32
BF16 = mybir.dt.bfloat16
AF = mybir.ActivationFunctionType
ALU = mybir.AluOpType

D = 512
H = 8
DH = 64
S_OWN = 1024   # query rows per core
S_OTH = 4096   # key/value rows (full opposite side)
NC = 8

LN_EPS = 1e-5
L2_EPS2 = 1e-24          # eps^2 for l2 norm (ref: max(norm, 1e-12))
QSCALE_LOG = float(np.log(0.125))  # DH ** -0.5 folded into inv-norm of q


def build_nc():
    nc = bacc.Bacc("TRN2", target_bir_lowering=False, debug=False, num_devices=NC)

    # ---- DRAM I/O ----
    x16_own = nc.dram_tensor("x16_own", [S_OWN, D], BF16, kind="ExternalInput")
    x16_oth = nc.dram_tensor("x16_oth", [S_OTH, D], BF16, kind="ExternalInput")
    xf_own = nc.dram_tensor("xf_own", [S_OWN, D], F32, kind="ExternalInput")
    wq_d = nc.dram_tensor("wq", [D, D], F32, kind="ExternalInput")
    wk_d = nc.dram_tensor("wk", [D, D], F32, kind="ExternalInput")
    wv_d = nc.dram_tensor("wv", [D, D], F32, kind="ExternalInput")
    wo_d = nc.dram_tensor("wo", [D, D], F32, kind="ExternalInput")
    wg_d = nc.dram_tensor("wg", [2 * D, D], F32, kind="ExternalInput")
    bq_d = nc.dram_tensor("bq", [D], F32, kind="ExternalInput")
    bk_d = nc.dram_tensor("bk", [D], F32, kind="ExternalInput")
    bv_d = nc.dram_tensor("bv", [D], F32, kind="ExternalInput")
    bo_d = nc.dram_tensor("bo", [D], F32, kind="ExternalInput")
    bg_d = nc.dram_tensor("bg", [D], F32, kind="ExternalInput")
    g_own_d = nc.dram_tensor("g_own", [D], F32, kind="ExternalInput")
    b_own_d = nc.dram_tensor("b_own", [D], F32, kind="ExternalInput")
    g_oth_d = nc.dram_tensor("g_oth", [D], F32, kind="ExternalInput")
    b_oth_d = nc.dram_tensor("b_oth", [D], F32, kind="ExternalInput")
    gg_d = nc.dram_tensor("gg", [D], F32, kind="ExternalInput")
    gb_d = nc.dram_tensor("gb", [D], F32, kind="ExternalInput")
    out_d = nc.dram_tensor("out", [S_OWN, D], F32, kind="ExternalOutput")

    # DRAM scratch for row replication round-trips
    scr_stats = {}
    for side, s in (("own", S_OWN), ("oth", S_OTH)):
        scr_stats[side] = {
            "mu": nc.dram_tensor(f"scr_mu_{side}", [s], F32),
            "m2": nc.dram_tensor(f"scr_m2_{side}", [s], F32),
            "negmu": nc.dram_tensor(f"scr_negmu_{side}", [s], BF16),
            "rstd": nc.dram_tensor(f"scr_rstd_{side}", [s], BF16),
            "invr": nc.dram_tensor(f"scr_invr_{side}", [s], BF16),
        }
    scr_ssq_q = nc.dram_tensor("scr_ssq_q", [H, S_OWN], F32)
    scr_ssq_k = nc.dram_tensor("scr_ssq_k", [H, S_OTH], F32)
    scr_inv_q = nc.dram_tensor("scr_inv_q", [H, S_OWN], BF16)
    scr_inv_k = nc.dram_tensor("scr_inv_k", [H, S_OTH], BF16)
    scr_rsum = nc.dram_tensor("scr_rsum", [H, S_OWN], BF16)

    def bcast_ap(dram, offset, nrep, n):
        return bass.AP(tensor=dram, offset=offset, ap=[[0, nrep], [1, n]])

    with tile.TileContext(nc) as tc:
        with tc.tile_pool(name="persist", bufs=1) as persist:
            # ---- constants ----
            ones_col = persist.tile([128, 1], BF16)       # K=128->M=1 colsum
            nc.vector.memset(ones_col, 1.0)
            oD_col = persist.tile([128, 1], BF16)         # 1/D for mean
            nc.vector.memset(oD_col, 1.0 / D)
            ones_row = persist.tile([1, 128], BF16)       # K=1 lhsT for bias outer
            nc.vector.memset(ones_row, 1.0)
            hsel = persist.tile([128, 2], BF16)           # per-head-pair colsum
            nc.vector.memset(hsel, 0.0)
            nc.vector.memset(hsel[0:64, 0:1], 1.0)
            nc.vector.memset(hsel[64:128, 1:2], 1.0)
            eps_col = persist.tile([128, 1], F32)
            nc.vector.memset(eps_col, LN_EPS)
            eps24_col = persist.tile([128, 1], F32)
            nc.vector.memset(eps24_col, L2_EPS2)
            qlog_col = persist.tile([128, 1], F32)
            nc.vector.memset(qlog_col, QSCALE_LOG)
            zero_col = persist.tile([128, 1], F32)
            nc.vector.memset(zero_col, 0.0)

            # ---- persistent SBUF tensors ----
            xT_own = persist.tile([128, 4, S_OWN], BF16)
            qT = persist.tile([128, 4, S_OWN], BF16)
            kT = persist.tile([128, 4, S_OTH], BF16)
            vsb = persist.tile([128, 32, H, DH + 1], BF16)
            ctxT = persist.tile([128, 4, S_OWN], BF16)
            wqb = persist.tile([128, 4, D], BF16)
            wkb = persist.tile([128, 4, D], BF16)
            wvb = persist.tile([128, 4, D], BF16)
            wob = persist.tile([128, 4, D], BF16)
            wgb = persist.tile([128, 8, D], BF16)
            csum_q = persist.tile([1, D], BF16)
            csum_k = persist.tile([1, D], BF16)
            bp_q = persist.tile([1, D], BF16)
            bp_k = persist.tile([1, D], BF16)
            bp_v = persist.tile([1, D], BF16)
            bo_row = persist.tile([1, D], BF16)
            bg_row = persist.tile([1, D], BF16)
            gg_rep = persist.tile([128, D], F32)
            gb_rep = persist.tile([128, D], F32)
            negmu = {"own": persist.tile([1, S_OWN], BF16, name="negmu_own"),
                     "oth": persist.tile([1, S_OTH], BF16, name="negmu_oth")}
            invr = {"own": persist.tile([1, S_OWN], BF16, name="invr_own"),
                    "oth": persist.tile([1, S_OTH], BF16, name="invr_oth")}
            rstd_rep_own = persist.tile([128, S_OWN], BF16)

            # ================= P1: weight prep =================
            with tc.tile_pool(name="wstage", bufs=2) as wstage, \
                 tc.tile_pool(name="wpsum", bufs=2, space="PSUM") as wpsum:
                gcols = {}
                for nm, dram in (("g_own", g_own_d), ("b_own", b_own_d),
                                 ("g_oth", g_oth_d), ("b_oth", b_oth_d)):
                    t = wstage.tile([128, 4], F32, tag=f"gcol_{nm}")
                    nc.sync.dma_start(out=t[:, :],
                                      in_=dram.ap().rearrange("(c p) -> p c", p=128))
                    gcols[nm] = t

                def prep_qk(w_dram, b_dram, g_nm, bln_nm, wb, csum, bp):
                    wst = wstage.tile([128, 4, D], F32, tag="wst")
                    nc.sync.dma_start(out=wst[:, :, :],
                                      in_=w_dram.ap().rearrange("(c p) d -> p c d", p=128))
                    for c in range(4):
                        nc.vector.tensor_scalar_mul(out=wb[:, c, :], in0=wst[:, c, :],
                                                    scalar1=gcols[g_nm][:, c:c + 1])
                    ps = wpsum.tile([1, D], F32, tag="wps")
                    for c in range(4):
                        nc.tensor.matmul(ps[:, :], ones_col[:, :], wb[:, c, :],
                                         start=(c == 0), stop=(c == 3))
                    nc.vector.tensor_copy(out=csum[:, :], in_=ps[:, :])
                    ps2 = wpsum.tile([1, D], F32, tag="wps")
                    for c in range(4):
                        nc.tensor.matmul(ps2[:, :], gcols[bln_nm][:, c:c + 1],
                                         wst[:, c, :], start=(c == 0), stop=(c == 3))
                    brow = wstage.tile([1, D], F32, tag="brow")
                    nc.sync.dma_start(out=brow[:, :], in_=b_dram.ap()[None, :])
                    bsum = wstage.tile([1, D], F32, tag="bsum")
                    nc.vector.tensor_add(out=bsum[:, :], in0=ps2[:, :], in1=brow[:, :])
                    nc.vector.tensor_copy(out=bp[:, :], in_=bsum[:, :])

                prep_qk(wq_d, bq_d, "g_own", "b_own", wqb, csum_q, bp_q)
                prep_qk(wk_d, bk_d, "g_oth", "b_oth", wkb, csum_k, bp_k)

                for w_dram, wb in ((wv_d, wvb), (wo_d, wob)):
                    wst = wstage.tile([128, 4, D], F32, tag="wst")
                    nc.sync.dma_start(out=wst[:, :, :],
                                      in_=w_dram.ap().rearrange("(c p) d -> p c d", p=128))
                    for c in range(4):
                        nc.vector.tensor_copy(out=wb[:, c, :], in_=wst[:, c, :])
                wst8 = wstage.tile([128, 8, D], F32, tag="wst8")
                nc.sync.dma_start(out=wst8[:, :, :],
                                  in_=wg_d.ap().rearrange("(c p) d -> p c d", p=128))
                for c in range(8):
                    nc.vector.tensor_copy(out=wgb[:, c, :], in_=wst8[:, c, :])

                for b_dram, row in ((bv_d, bp_v), (bo_d, bo_row), (bg_d, bg_row)):
                    br = wstage.tile([1, D], F32, tag="brow")
                    nc.sync.dma_start(out=br[:, :], in_=b_dram.ap()[None, :])
                    nc.vector.tensor_copy(out=row[:, :], in_=br[:, :])

                for dram, rep in ((gg_d, gg_rep), (gb_d, gb_rep)):
                    nc.sync.dma_start(out=rep[:, :], in_=bcast_ap(dram, 0, 128, D))

            # ================= P2-P5 (need xT_oth alive) =================
            xT_oth_cm = tc.tile_pool(name="xT_oth", bufs=1)
            xT_oth_pool = xT_oth_cm.__enter__()
            xT_oth = xT_oth_pool.tile([128, 4, S_OTH], BF16)
            for c in range(4):
                nc.sync.dma_start_transpose(out=xT_own[:, c, :],
                                            in_=x16_own.ap()[:, c * 128:(c + 1) * 128])
                nc.sync.dma_start_transpose(out=xT_oth[:, c, :],
                                            in_=x16_oth.ap()[:, c * 128:(c + 1) * 128])

            # ---- P3: LN stats ----
            with tc.tile_pool(name="sq", bufs=2) as sqp, \
                 tc.tile_pool(name="stps", bufs=2, space="PSUM") as stps:
                for side, s, xt in (("own", S_OWN, xT_own), ("oth", S_OTH, xT_oth)):
                    nwin = s // 512
                    for w in range(nwin):
                        wsl = slice(w * 512, (w + 1) * 512)
                        ps_mu = stps.tile([1, 512], F32, tag="psmu")
                        ps_m2 = stps.tile([1, 512], F32, tag="psm2")
                        for c in range(4):
                            sq = sqp.tile([128, 512], BF16, tag="sq")
                            nc.vector.tensor_mul(out=sq[:, :], in0=xt[:, c, wsl],
                                                 in1=xt[:, c, wsl])
                            nc.tensor.matmul(ps_mu[:, :], oD_col[:, :], xt[:, c, wsl],
                                             start=(c == 0), stop=(c == 3))
                            nc.tensor.matmul(ps_m2[:, :], oD_col[:, :], sq[:, :],
                                             start=(c == 0), stop=(c == 3))
                        strow_mu = sqp.tile([1, 512], F32, tag="strow_mu")
                        strow_m2 = sqp.tile([1, 512], F32, tag="strow_m2")
                        nc.vector.tensor_copy(out=strow_mu[:, :], in_=ps_mu[:, :])
                        nc.vector.tensor_copy(out=strow_m2[:, :], in_=ps_m2[:, :])
                        nc.sync.dma_start(
                            out=scr_stats[side]["mu"].ap()[wsl][None, :],
                            in_=strow_mu[:, :])
                        nc.sync.dma_start(
                            out=scr_stats[side]["m2"].ap()[wsl][None, :],
                            in_=strow_m2[:, :])
                for side, s in (("own", S_OWN), ("oth", S_OTH)):
                    fcol = s // 128
                    mu_pk = sqp.tile([128, fcol], F32, tag="pk_mu")
                    m2_pk = sqp.tile([128, fcol], F32, tag="pk_m2")
                    nc.sync.dma_start(
                        out=mu_pk[:, :],
                        in_=scr_stats[side]["mu"].ap().rearrange("(p f) -> p f", p=128))
                    nc.sync.dma_start(
                        out=m2_pk[:, :],
                        in_=scr_stats[side]["m2"].ap().rearrange("(p f) -> p f", p=128))
                    msq = sqp.tile([128, fcol], F32, tag="pk_msq")
                    nc.scalar.activation(out=msq[:, :], in_=mu_pk[:, :], func=AF.Square)
                    var = sqp.tile([128, fcol], F32, tag="pk_var")
                    nc.vector.tensor_sub(out=var[:, :], in0=m2_pk[:, :], in1=msq[:, :])
                    lnv = sqp.tile([128, fcol], F32, tag="pk_lnv")
                    nc.scalar.activation(out=lnv[:, :], in_=var[:, :], func=AF.Ln,
                                         bias=eps_col[:, :])
                    rstd_pk = sqp.tile([128, fcol], BF16, tag="pk_rstd")
                    nc.scalar.activation(out=rstd_pk[:, :], in_=lnv[:, :], func=AF.Exp,
                                         scale=-0.5)
                    invr_pk = sqp.tile([128, fcol], BF16, tag="pk_invr")
                    nc.scalar.activation(out=invr_pk[:, :], in_=lnv[:, :], func=AF.Exp,
                                         scale=0.5)
                    nmu_pk = sqp.tile([128, fcol], BF16, tag="pk_nmu")
                    nc.vector.tensor_scalar_mul(out=nmu_pk[:, :], in0=mu_pk[:, :],
                                                scalar1=-1.0)
                    for nm, pk in (("negmu", nmu_pk), ("rstd", rstd_pk),
                                   ("invr", invr_pk)):
                        nc.sync.dma_start(
                            out=scr_stats[side][nm].ap().rearrange("(p f) -> p f", p=128),
                            in_=pk[:, :])
                for side in ("own", "oth"):
                    nc.sync.dma_start(out=negmu[side][:, :],
                                      in_=scr_stats[side]["negmu"].ap()[None, :])
                    nc.sync.dma_start(out=invr[side][:, :],
                                      in_=scr_stats[side]["invr"].ap()[None, :])
                nc.sync.dma_start(out=rstd_rep_own[:, :],
                                  in_=bcast_ap(scr_stats["own"]["rstd"], 0, 128, S_OWN))

            # ---- P4: projections ----
            with tc.tile_pool(name="prps", bufs=2, space="PSUM") as prps, \
                 tc.tile_pool(name="prtmp", bufs=1) as prtmp:
                rstd_rep_oth = prtmp.tile([128, S_OTH], BF16, tag="rstdoth")
                nc.sync.dma_start(out=rstd_rep_oth[:, :],
                                  in_=bcast_ap(scr_stats["oth"]["rstd"], 0, 128, S_OTH))
                for oc in range(4):
                    osl = slice(oc * 128, (oc + 1) * 128)
                    for w in range(S_OWN // 512):
                        wsl = slice(w * 512, (w + 1) * 512)
                        ps = prps.tile([128, 512], F32, tag="pps")
                        for c in range(4):
                            nc.tensor.matmul(ps[:, :], wqb[:, c, osl],
                                             xT_own[:, c, wsl],
                                             start=(c == 0), stop=False)
                        nc.tensor.matmul(ps[:, :], csum_q[:, osl],
                                         negmu["own"][:, wsl], start=False, stop=False)
                        nc.tensor.matmul(ps[:, :], bp_q[:, osl],
                                         invr["own"][:, wsl], start=False, stop=True)
                        nc.vector.tensor_mul(out=qT[:, oc, wsl], in0=ps[:, :],
                                             in1=rstd_rep_own[:, wsl])
                for oc in range(4):
                    osl = slice(oc * 128, (oc + 1) * 128)
                    for w in range(S_OTH // 512):
                        wsl = slice(w * 512, (w + 1) * 512)
                        ps = prps.tile([128, 512], F32, tag="pps")
                        for c in range(4):
                            nc.tensor.matmul(ps[:, :], wkb[:, c, osl],
                                             xT_oth[:, c, wsl],
                                             start=(c == 0), stop=False)
                        nc.tensor.matmul(ps[:, :], csum_k[:, osl],
                                         negmu["oth"][:, wsl], start=False, stop=False)
                        nc.tensor.matmul(ps[:, :], bp_k[:, osl],
                                         invr["oth"][:, wsl], start=False, stop=True)
                        nc.vector.tensor_mul(out=kT[:, oc, wsl], in0=ps[:, :],
                                             in1=rstd_rep_oth[:, wsl])
                for sb in range(32):
                    ps = prps.tile([128, D], F32, tag="vps")
                    ssl = slice(sb * 128, (sb + 1) * 128)
                    for c in range(4):
                        nc.tensor.matmul(ps[:, :], xT_oth[:, c, ssl], wvb[:, c, :],
                                         start=(c == 0), stop=False)
                    nc.tensor.matmul(ps[:, :], ones_row[:, :], bp_v[:, :],
                                     start=False, stop=True)
                    nc.vector.tensor_copy(
                        out=vsb[:, sb, :, 0:DH],
                        in_=ps[:, :].rearrange("p (h d) -> p h d", h=H))
                nc.vector.memset(vsb[:, :, :, DH:DH + 1], 1.0)

            xT_oth_cm.__exit__(None, None, None)

            # ---- P5: l2 normalization of q/k ----
            with tc.tile_pool(name="l2", bufs=2) as l2p, \
                 tc.tile_pool(name="l2ps", bufs=2, space="PSUM") as l2ps:
                for (name, t, s, scr_ssq, scr_inv) in (
                        ("q", qT, S_OWN, scr_ssq_q, scr_inv_q),
                        ("k", kT, S_OTH, scr_ssq_k, scr_inv_k)):
                    nwin = s // 512
                    for oc in range(4):
                        for w in range(nwin):
                            wsl = slice(w * 512, (w + 1) * 512)
                            sq = l2p.tile([128, 512], BF16, tag="l2sq")
                            nc.vector.tensor_mul(out=sq[:, :], in0=t[:, oc, wsl],
                                                 in1=t[:, oc, wsl])
                            ps = l2ps.tile([2, 512], F32, tag="l2ps")
                            nc.tensor.matmul(ps[:, :], hsel[:, :], sq[:, :],
                                             start=True, stop=True)
                            ssrow = l2p.tile([2, 512], F32, tag="ssrow")
                            nc.vector.tensor_copy(out=ssrow[:, :], in_=ps[:, :])
                            nc.sync.dma_start(out=scr_ssq.ap()[2 * oc:2 * oc + 1, wsl],
                                              in_=ssrow[0:1, :])
                            nc.sync.dma_start(
                                out=scr_ssq.ap()[2 * oc + 1:2 * oc + 2, wsl],
                                in_=ssrow[1:2, :])
                    fcol = H * s // 128
                    pk = l2p.tile([128, fcol], F32, tag=f"l2pk_{name}")
                    nc.sync.dma_start(
                        out=pk[:, :].rearrange("p (h f) -> p h f", h=H),
                        in_=scr_ssq.ap().rearrange("h (p f) -> p h f", p=128))
                    lns = l2p.tile([128, fcol], F32, tag=f"l2ln_{name}")
                    nc.scalar.activation(out=lns[:, :], in_=pk[:, :], func=AF.Ln,
                                         bias=eps24_col[:, :])
                    ipk = l2p.tile([128, fcol], BF16, tag=f"l2in_{name}")
                    nc.scalar.activation(out=ipk[:, :], in_=lns[:, :], func=AF.Exp,
                                         scale=-0.5,
                                         bias=(qlog_col[:, :] if name == "q" else zero_col[:, :]))
                    nc.sync.dma_start(
                        out=scr_inv.ap().rearrange("h (p f) -> p h f", p=128),
                        in_=ipk[:, :].rearrange("p (h f) -> p h f", h=H))
                    for oc in range(4):
                        rep = l2p.tile([128, S_OTH], BF16, name="l2rep", tag="l2rep")[:, :s]
                        nc.sync.dma_start(out=rep[0:64, :],
                                          in_=bcast_ap(scr_inv, (2 * oc) * s, 64, s))
                        nc.sync.dma_start(out=rep[64:128, :],
                                          in_=bcast_ap(scr_inv, (2 * oc + 1) * s, 64, s))
                        nc.vector.tensor_mul(out=t[:, oc, :], in0=t[:, oc, :],
                                             in1=rep[:, :])

            # ================= P6: attention =================
            with tc.tile_pool(name="scps", bufs=2, space="PSUM") as scps, \
                 tc.tile_pool(name="ctps", bufs=2, space="PSUM") as ctps, \
                 tc.tile_pool(name="att", bufs=4) as attp, \
                 tc.tile_pool(name="attr", bufs=2) as attrp:
                for qw in range(2):
                    qsl = slice(qw * 512, (qw + 1) * 512)
                    for hp in range(4):
                        ctx2 = [ctps.tile([DH + 1, 512], F32, name=f"ctx{j}",
                                          tag=f"ctx{j}")
                                for j in range(2)]
                        for kc in range(32):
                            ksl = slice(kc * 128, (kc + 1) * 128)
                            for j in range(2):
                                psl = slice(64 * j, 64 * (j + 1))
                                sc = scps.tile([128, 512], F32, tag=f"sc{j}")
                                nc.tensor.matmul(sc[:, :], kT[psl, hp, ksl],
                                                 qT[psl, hp, qsl],
                                                 start=True, stop=True)
                                e = attp.tile([128, 512], BF16, tag=f"e{j}")
                                nc.scalar.activation(out=e[:, :], in_=sc[:, :],
                                                     func=AF.Exp)
                                nc.tensor.matmul(ctx2[j][:, :],
                                                 vsb[:, kc, 2 * hp + j, :], e[:, :],
                                                 start=(kc == 0), stop=(kc == 31))
                        for j in range(2):
                            h = 2 * hp + j
                            rs = attrp.tile([DH + 1, 512], F32, tag="rs")
                            nc.vector.reciprocal(out=rs[DH:DH + 1, :],
                                                 in_=ctx2[j][DH:DH + 1, :])
                            rs16 = attrp.tile([DH + 1, 512], BF16, tag="rs16")
                            nc.vector.tensor_copy(out=rs16[DH:DH + 1, :],
                                                  in_=rs[DH:DH + 1, :])
                            nc.sync.dma_start(out=scr_rsum.ap()[h:h + 1, qsl],
                                              in_=rs16[DH:DH + 1, :])
                            rep = attrp.tile([64, 512], BF16, tag="rsrep")
                            nc.sync.dma_start(
                                out=rep[:, :],
                                in_=bcast_ap(scr_rsum, h * S_OWN + qw * 512, 64, 512))
                            nc.vector.tensor_mul(
                                out=ctxT[64 * j:64 * (j + 1), hp, qsl],
                                in0=ctx2[j][0:DH, :], in1=rep[:, :])

            # ================= P7: output proj + gate + residual =================
            with tc.tile_pool(name="ops", bufs=2, space="PSUM") as opsp, \
                 tc.tile_pool(name="fin", bufs=3) as finp:
                for sb in range(8):
                    ssl = slice(sb * 128, (sb + 1) * 128)
                    ps_o = opsp.tile([128, D], F32, tag="pso")
                    for c in range(4):
                        nc.tensor.matmul(ps_o[:, :], ctxT[:, c, ssl], wob[:, c, :],
                                         start=(c == 0), stop=False)
                    nc.tensor.matmul(ps_o[:, :], ones_row[:, :], bo_row[:, :],
                                     start=False, stop=True)
                    proj = finp.tile([128, D], F32, tag="proj")
                    nc.vector.tensor_copy(out=proj[:, :], in_=ps_o[:, :])

                    ps_z = opsp.tile([128, D], F32, tag="psz")
                    for c in range(4):
                        nc.tensor.matmul(ps_z[:, :], ctxT[:, c, ssl], wgb[:, c, :],
                                         start=(c == 0), stop=False)
                    for c in range(4):
                        nc.tensor.matmul(ps_z[:, :], xT_own[:, c, ssl],
                                         wgb[:, 4 + c, :], start=False, stop=False)
                    nc.tensor.matmul(ps_z[:, :], ones_row[:, :], bg_row[:, :],
                                     start=False, stop=True)
                    z = finp.tile([128, D], F32, tag="z")
                    nc.vector.tensor_copy(out=z[:, :], in_=ps_z[:, :])
                    stats = finp.tile([128, 6], F32, tag="st6")
                    nc.vector.bn_stats(out=stats[:, :], in_=z[:, :])
                    mv = finp.tile([128, 2], F32, tag="mv")
                    nc.vector.bn_aggr(out=mv[:, :], in_=stats[:, :])
                    lnv = finp.tile([128, 1], F32, tag="lnv")
                    nc.scalar.activation(out=lnv[:, :], in_=mv[:, 1:2], func=AF.Ln,
                                         bias=eps_col[:, :])
                    rstd = finp.tile([128, 1], F32, tag="rstd")
                    nc.scalar.activation(out=rstd[:, :], in_=lnv[:, :], func=AF.Exp,
                                         scale=-0.5)
                    zn = finp.tile([128, D], F32, tag="zn")
                    nc.vector.tensor_scalar(out=zn[:, :], in0=z[:, :],
                                            scalar1=mv[:, 0:1], scalar2=rstd[:, :],
                                            op0=ALU.subtract, op1=ALU.mult)
                    zg = finp.tile([128, D], F32, tag="zg")
                    nc.vector.tensor_mul(out=zg[:, :], in0=zn[:, :], in1=gg_rep[:, :])
                    nc.vector.tensor_add(out=zg[:, :], in0=zg[:, :], in1=gb_rep[:, :])
                    ex = finp.tile([128, D], F32, tag="ex")
                    nc.scalar.activation(out=ex[:, :], in_=zg[:, :], func=AF.Exp,
                                         scale=-1.0)
                    nc.vector.tensor_scalar_add(out=ex[:, :], in0=ex[:, :], scalar1=1.0)
                    gate = finp.tile([128, D], F32, tag="gate")
                    nc.vector.reciprocal(out=gate[:, :], in_=ex[:, :])

                    xblk = finp.tile([128, D], F32, tag="xblk")
                    nc.sync.dma_start(out=xblk[:, :], in_=xf_own.ap()[ssl, :])
                    gp = finp.tile([128, D], F32, tag="gp")
                    nc.vector.tensor_mul(out=gp[:, :], in0=gate[:, :], in1=proj[:, :])
                    ob = finp.tile([128, D], F32, tag="ob")
                    nc.vector.tensor_add(out=ob[:, :], in0=gp[:, :], in1=xblk[:, :])
                    nc.sync.dma_start(out=out_d.ap()[ssl, :], in_=ob[:, :])

    nc.compile()
    return nc


_NC_CACHE = None


def _get_nc():
    global _NC_CACHE
    if _NC_CACHE is None:
        _NC_CACHE = build_nc()
    return _NC_CACHE


def make_in_maps(inputs):
    xg = np.ascontiguousarray(np.asarray(inputs["gene_embeds"], np.float32))
    xd = np.ascontiguousarray(np.asarray(inputs["drug_embeds"], np.float32))
    xg16 = xg.astype(ml_dtypes.bfloat16)
    xd16 = xd.astype(ml_dtypes.bfloat16)

    f32 = lambda k: np.ascontiguousarray(np.asarray(inputs[k], np.float32))

    gene_common = dict(
        x16_oth=xd16, wq=f32("wgq"), wk=f32("wdk"), wv=f32("wdv"), wo=f32("wo"),
        wg=f32("wgg"), bq=f32("bgq"), bk=f32("bdk"), bv=f32("bdv"), bo=f32("bo"),
        bg=f32("bgg"), g_own=f32("lng_g"), b_own=f32("lng_b"), g_oth=f32("lnd_g"),
        b_oth=f32("lnd_b"), gg=f32("gg_g"), gb=f32("gg_b"))
    drug_common = dict(
        x16_oth=xg16, wq=f32("wdq"), wk=f32("wgk"), wv=f32("wgv"), wo=f32("wo"),
        wg=f32("wdg"), bq=f32("bdq"), bk=f32("bgk"), bv=f32("bgv"), bo=f32("bo"),
        bg=f32("bdg"), g_own=f32("lnd_g"), b_own=f32("lnd_b"), g_oth=f32("lng_g"),
        b_oth=f32("lng_b"), gg=f32("dg_g"), gb=f32("dg_b"))

    in_maps = []
    for i in range(8):
        if i < 4:
            sl = slice(i * S_OWN, (i + 1) * S_OWN)
            m = dict(gene_common)
            m["x16_own"] = np.ascontiguousarray(xg16[sl])
            m["xf_own"] = np.ascontiguousarray(xg[sl])
        else:
            sl = slice((i - 4) * S_OWN, (i - 3) * S_OWN)
            m = dict(drug_common)
            m["x16_own"] = np.ascontiguousarray(xd16[sl])
            m["xf_own"] = np.ascontiguousarray(xd[sl])
        in_maps.append(m)
    return in_maps


def kernel(**inputs):
    nc = _get_nc()
    in_maps = make_in_maps(inputs)
    res = run_bass_kernel_spmd(nc, in_maps, core_ids=list(range(8)))
    gene_out = np.concatenate([res.results[i]["out"] for i in range(4)], axis=0)
    drug_out = np.concatenate([res.results[i]["out"] for i in range(4, 8)], axis=0)
    return (gene_out, drug_out)


# revision 30
# speedup vs baseline: 4321.2456x; 1.1651x over previous
"""Trainium2 Bass kernel for EnhancedCrossAttention (dense transformer, 8-core SPMD).

Sharding: cores 0-3 compute gene_out rows [1024*i, 1024*(i+1)) attending over all
drug K/V; cores 4-7 mirror for drug_out. One SPMD program with direction-generic
input names; host slices/replicates inputs and concatenates outputs.

Layout strategy: all activations live transposed [dims, seq] on-chip (loaded via
DMA-xbar transpose); LayerNorm is folded into the projection matmuls as rank-1
PSUM corrections, so q/k/v projections consume the raw transposed embeds
directly and produce qT/kT ready for the score matmuls with no PE transposes.
Scores are computed transposed [k, q] so the exp'd tile is directly the lhsT of
the context matmul; a ones-column in V yields softmax denominators for free.
"""
import numpy as np
import ml_dtypes

import concourse.bass as bass
import concourse.mybir as mybir
import concourse.tile as tile
from concourse import bacc
from concourse.bass_utils import run_bass_kernel_spmd

F32 = mybir.dt.float32
BF16 = mybir.dt.bfloat16
AF = mybir.ActivationFunctionType
ALU = mybir.AluOpType

D = 512
H = 8
DH = 64
S_OWN = 1024   # query rows per core
S_OTH = 4096   # key/value rows (full opposite side)
NC = 8

LN_EPS = 1e-5
L2_EPS2 = 1e-24          # eps^2 for l2 norm (ref: max(norm, 1e-12))
QSCALE_LOG = float(np.log(0.125))  # DH ** -0.5 folded into inv-norm of q


def build_nc():
    nc = bacc.Bacc("TRN2", target_bir_lowering=False, debug=False, num_devices=NC)

    # ---- DRAM I/O ----
    x16_own = nc.dram_tensor("x16_own", [S_OWN, D], BF16, kind="ExternalInput")
    x16_oth = nc.dram_tensor("x16_oth", [S_OTH, D], BF16, kind="ExternalInput")
    xf_own = nc.dram_tensor("xf_own", [S_OWN, D], F32, kind="ExternalInput")
    wq_d = nc.dram_tensor("wq", [D, D], F32, kind="ExternalInput")
    wk_d = nc.dram_tensor("wk", [D, D], F32, kind="ExternalInput")
    wv_d = nc.dram_tensor("wv", [D, D], F32, kind="ExternalInput")
    wo_d = nc.dram_tensor("wo", [D, D], F32, kind="ExternalInput")
    wg_d = nc.dram_tensor("wg", [2 * D, D], F32, kind="ExternalInput")
    bq_d = nc.dram_tensor("bq", [D], F32, kind="ExternalInput")
    bk_d = nc.dram_tensor("bk", [D], F32, kind="ExternalInput")
    bv_d = nc.dram_tensor("bv", [D], F32, kind="ExternalInput")
    bo_d = nc.dram_tensor("bo", [D], F32, kind="ExternalInput")
    bg_d = nc.dram_tensor("bg", [D], F32, kind="ExternalInput")
    g_own_d = nc.dram_tensor("g_own", [D], F32, kind="ExternalInput")
    b_own_d = nc.dram_tensor("b_own", [D], F32, kind="ExternalInput")
    g_oth_d = nc.dram_tensor("g_oth", [D], F32, kind="ExternalInput")
    b_oth_d = nc.dram_tensor("b_oth", [D], F32, kind="ExternalInput")
    gg_d = nc.dram_tensor("gg", [D], F32, kind="ExternalInput")
    gb_d = nc.dram_tensor("gb", [D], F32, kind="ExternalInput")
    out_d = nc.dram_tensor("out", [S_OWN, D], F32, kind="ExternalOutput")

    # DRAM scratch for row replication round-trips
    scr_stats = {}
    for side, s in (("own", S_OWN), ("oth", S_OTH)):
        scr_stats[side] = {
            "mu": nc.dram_tensor(f"scr_mu_{side}", [s], F32),
            "m2": nc.dram_tensor(f"scr_m2_{side}", [s], F32),
            "negmu": nc.dram_tensor(f"scr_negmu_{side}", [s], BF16),
            "rstd": nc.dram_tensor(f"scr_rstd_{side}", [s], BF16),
            "invr": nc.dram_tensor(f"scr_invr_{side}", [s], BF16),
        }
    scr_ssq_q = nc.dram_tensor("scr_ssq_q", [H, S_OWN], F32)
    scr_ssq_k = nc.dram_tensor("scr_ssq_k", [H, S_OTH], F32)
    scr_inv_q = nc.dram_tensor("scr_inv_q", [H, S_OWN], BF16)
    scr_inv_k = nc.dram_tensor("scr_inv_k", [H, S_OTH], BF16)
    scr_rsum = nc.dram_tensor("scr_rsum", [H, S_OWN], BF16)

    def bcast_ap(dram, offset, nrep, n):
        return bass.AP(tensor=dram, offset=offset, ap=[[0, nrep], [1, n]])

    I32 = mybir.dt.int32
    MAGIC = 0x5F3759DF

    def rsqrt_dve(nc, pool, x, tag, eps=0.0, newton=2, out_dtype=F32,
                  post_scale=None):
        """out = post_scale * 1/sqrt(x + eps), all on DVE (no ACT tables)."""
        p, f = x.shape[0], x.free_size()
        xe = pool.tile([p, f], F32, name=f"{tag}_xe", tag=f"{tag}_xe")
        if eps:
            nc.vector.tensor_scalar_add(out=xe[:, :], in0=x, scalar1=float(eps))
        else:
            nc.vector.tensor_copy(out=xe[:, :], in_=x)
        it = pool.tile([p, f], I32, name=f"{tag}_it", tag=f"{tag}_it")
        nc.vector.tensor_scalar(out=it[:, :], in0=xe[:, :].bitcast(I32),
                                scalar1=1, scalar2=None,
                                op0=ALU.arith_shift_right)
        nc.vector.tensor_scalar(out=it[:, :], in0=it[:, :],
                                scalar1=-1, scalar2=MAGIC,
                                op0=ALU.mult, op1=ALU.add)
        y = pool.tile([p, f], F32, name=f"{tag}_y", tag=f"{tag}_y")
        nc.vector.tensor_copy(out=y[:, :], in_=it[:, :].bitcast(F32))
        t1 = pool.tile([p, f], F32, name=f"{tag}_t1", tag=f"{tag}_t1")
        for _ in range(newton):
            nc.vector.tensor_mul(out=t1[:, :], in0=y[:, :], in1=y[:, :])
            nc.vector.tensor_mul(out=t1[:, :], in0=t1[:, :], in1=xe[:, :])
            nc.vector.tensor_scalar(out=t1[:, :], in0=t1[:, :],
                                    scalar1=-0.5, scalar2=1.5,
                                    op0=ALU.mult, op1=ALU.add)
            nc.vector.tensor_mul(out=y[:, :], in0=y[:, :], in1=t1[:, :])
        out = pool.tile([p, f], out_dtype, name=f"{tag}_o", tag=f"{tag}_o")
        if post_scale is not None:
            nc.vector.tensor_scalar_mul(out=out[:, :], in0=y[:, :],
                                        scalar1=float(post_scale))
        else:
            nc.vector.tensor_copy(out=out[:, :], in_=y[:, :])
        return out, xe, y

    with tile.TileContext(nc) as tc:
        with tc.tile_pool(name="persist", bufs=1) as persist:
            # ---- constants ----
            ones_col = persist.tile([128, 1], BF16)       # K=128->M=1 colsum
            nc.vector.memset(ones_col, 1.0)
            oD_col = persist.tile([128, 1], BF16)         # 1/D for mean
            nc.vector.memset(oD_col, 1.0 / D)
            ones_row = persist.tile([1, 128], BF16)       # K=1 lhsT for bias outer
            nc.vector.memset(ones_row, 1.0)
            hsel = persist.tile([128, 2], BF16)           # per-head-pair colsum
            nc.vector.memset(hsel, 0.0)
            nc.vector.memset(hsel[0:64, 0:1], 1.0)
            nc.vector.memset(hsel[64:128, 1:2], 1.0)
            eps_col = persist.tile([128, 1], F32)
            nc.vector.memset(eps_col, LN_EPS)
            eps24_col = persist.tile([128, 1], F32)
            nc.vector.memset(eps24_col, L2_EPS2)
            qlog_col = persist.tile([128, 1], F32)
            nc.vector.memset(qlog_col, QSCALE_LOG)
            zero_col = persist.tile([128, 1], F32)
            nc.vector.memset(zero_col, 0.0)

            # ---- persistent SBUF tensors ----
            xT_own = persist.tile([128, 4, S_OWN], BF16)
            qT = persist.tile([128, 4, S_OWN], BF16)
            kT = persist.tile([128, 4, S_OTH], BF16)
            vsb = persist.tile([128, 16, 2, H, 72], mybir.dt.float8e4)
            ctxT = persist.tile([128, 4, S_OWN], BF16)
            wqb = persist.tile([128, 4, D], BF16)
            wkb = persist.tile([128, 4, D], BF16)
            wvb = persist.tile([128, 4, D], BF16)
            wob = persist.tile([128, 4, D], BF16)
            wgb = persist.tile([128, 8, D], BF16)
            csum_q = persist.tile([1, D], BF16)
            csum_k = persist.tile([1, D], BF16)
            bp_q = persist.tile([1, D], BF16)
            bp_k = persist.tile([1, D], BF16)
            bp_v = persist.tile([1, D], BF16)
            bo_row = persist.tile([1, D], BF16)
            bg_row = persist.tile([1, D], BF16)
            gg_rep = persist.tile([128, D], F32)
            gb_rep = persist.tile([128, D], F32)
            negmu = {"own": persist.tile([1, S_OWN], BF16, name="negmu_own"),
                     "oth": persist.tile([1, S_OTH], BF16, name="negmu_oth")}
            invr = {"own": persist.tile([1, S_OWN], BF16, name="invr_own"),
                    "oth": persist.tile([1, S_OTH], BF16, name="invr_oth")}

            # ================= P1: weight prep =================
            with tc.tile_pool(name="wstage", bufs=2) as wstage, \
                 tc.tile_pool(name="wpsum", bufs=2, space="PSUM") as wpsum:
                gcols = {}
                for nm, dram in (("g_own", g_own_d), ("b_own", b_own_d),
                                 ("g_oth", g_oth_d), ("b_oth", b_oth_d)):
                    t = wstage.tile([128, 4], F32, tag=f"gcol_{nm}")
                    nc.sync.dma_start(out=t[:, :],
                                      in_=dram.ap().rearrange("(c p) -> p c", p=128))
                    gcols[nm] = t

                def prep_qk(w_dram, b_dram, g_nm, bln_nm, wb, csum, bp):
                    wst = wstage.tile([128, 4, D], F32, tag="wst")
                    nc.sync.dma_start(out=wst[:, :, :],
                                      in_=w_dram.ap().rearrange("(c p) d -> p c d", p=128))
                    for c in range(4):
                        nc.vector.tensor_scalar_mul(out=wb[:, c, :], in0=wst[:, c, :],
                                                    scalar1=gcols[g_nm][:, c:c + 1])
                    ps = wpsum.tile([1, D], F32, tag="wps")
                    for c in range(4):
                        nc.tensor.matmul(ps[:, :], ones_col[:, :], wb[:, c, :],
                                         start=(c == 0), stop=(c == 3))
                    nc.vector.tensor_copy(out=csum[:, :], in_=ps[:, :])
                    ps2 = wpsum.tile([1, D], F32, tag="wps")
                    for c in range(4):
                        nc.tensor.matmul(ps2[:, :], gcols[bln_nm][:, c:c + 1],
                                         wst[:, c, :], start=(c == 0), stop=(c == 3))
                    brow = wstage.tile([1, D], F32, tag="brow")
                    nc.sync.dma_start(out=brow[:, :], in_=b_dram.ap()[None, :])
                    bsum = wstage.tile([1, D], F32, tag="bsum")
                    nc.vector.tensor_add(out=bsum[:, :], in0=ps2[:, :], in1=brow[:, :])
                    nc.vector.tensor_copy(out=bp[:, :], in_=bsum[:, :])

                prep_qk(wq_d, bq_d, "g_own", "b_own", wqb, csum_q, bp_q)
                prep_qk(wk_d, bk_d, "g_oth", "b_oth", wkb, csum_k, bp_k)

                for w_dram, wb in ((wv_d, wvb), (wo_d, wob)):
                    wst = wstage.tile([128, 4, D], F32, tag="wst")
                    nc.sync.dma_start(out=wst[:, :, :],
                                      in_=w_dram.ap().rearrange("(c p) d -> p c d", p=128))
                    for c in range(4):
                        nc.vector.tensor_copy(out=wb[:, c, :], in_=wst[:, c, :])
                wst8 = wstage.tile([128, 8, D], F32, tag="wst8")
                nc.sync.dma_start(out=wst8[:, :, :],
                                  in_=wg_d.ap().rearrange("(c p) d -> p c d", p=128))
                for c in range(8):
                    nc.vector.tensor_copy(out=wgb[:, c, :], in_=wst8[:, c, :])

                for b_dram, row in ((bv_d, bp_v), (bo_d, bo_row), (bg_d, bg_row)):
                    br = wstage.tile([1, D], F32, tag="brow")
                    nc.sync.dma_start(out=br[:, :], in_=b_dram.ap()[None, :])
                    nc.vector.tensor_copy(out=row[:, :], in_=br[:, :])

                for dram, rep in ((gg_d, gg_rep), (gb_d, gb_rep)):
                    nc.sync.dma_start(out=rep[:, :], in_=bcast_ap(dram, 0, 128, D))

            # ================= P2-P5 (need xT_oth alive) =================
            xT_oth_cm = tc.tile_pool(name="xT_oth", bufs=1)
            xT_oth_pool = xT_oth_cm.__enter__()
            xT_oth = xT_oth_pool.tile([128, 4, S_OTH], BF16)
            for c in range(4):
                nc.sync.dma_start_transpose(out=xT_own[:, c, :],
                                            in_=x16_own.ap()[:, c * 128:(c + 1) * 128])
                nc.sync.dma_start_transpose(out=xT_oth[:, c, :],
                                            in_=x16_oth.ap()[:, c * 128:(c + 1) * 128])

            # ---- P3: LN stats ----
            with tc.tile_pool(name="sq", bufs=2) as sqp, \
                 tc.tile_pool(name="stps", bufs=2, space="PSUM") as stps:
                for side, s, xt in (("own", S_OWN, xT_own), ("oth", S_OTH, xT_oth)):
                    nwin = s // 512
                    for w in range(nwin):
                        wsl = slice(w * 512, (w + 1) * 512)
                        ps_mu = stps.tile([1, 512], F32, tag="psmu", name="psmu")
                        ps_m2 = stps.tile([1, 512], F32, tag="psm2", name="psm2")
                        for c in range(4):
                            sq = sqp.tile([128, 512], BF16, tag="sq", name="sq")
                            nc.scalar.activation(out=sq[:, :], in_=xt[:, c, wsl],
                                                 func=AF.Square)
                            nc.tensor.matmul(ps_mu[:, :], oD_col[:, :], xt[:, c, wsl],
                                             start=(c == 0), stop=(c == 3))
                            nc.tensor.matmul(ps_m2[:, :], oD_col[:, :], sq[:, :],
                                             start=(c == 0), stop=(c == 3))
                        strow_mu = sqp.tile([1, 512], F32, tag="strow_mu",
                                            name="strow_mu")
                        strow_m2 = sqp.tile([1, 512], F32, tag="strow_m2",
                                            name="strow_m2")
                        nc.vector.tensor_copy(out=strow_mu[:, :], in_=ps_mu[:, :])
                        nc.vector.tensor_copy(out=strow_m2[:, :], in_=ps_m2[:, :])
                        nc.gpsimd.dma_start(
                            out=scr_stats[side]["mu"].ap()[wsl][None, :],
                            in_=strow_mu[:, :])
                        nc.gpsimd.dma_start(
                            out=scr_stats[side]["m2"].ap()[wsl][None, :],
                            in_=strow_m2[:, :])
                for side, s in (("own", S_OWN), ("oth", S_OTH)):
                    fcol = s // 128
                    mu_pk = sqp.tile([128, fcol], F32, tag="pk_mu", name="mu_pk")
                    m2_pk = sqp.tile([128, fcol], F32, tag="pk_m2", name="m2_pk")
                    nc.gpsimd.dma_start(
                        out=mu_pk[:, :],
                        in_=scr_stats[side]["mu"].ap().rearrange("(p f) -> p f", p=128))
                    nc.gpsimd.dma_start(
                        out=m2_pk[:, :],
                        in_=scr_stats[side]["m2"].ap().rearrange("(p f) -> p f", p=128))
                    msq = sqp.tile([128, fcol], F32, tag="pk_msq", name="msq")
                    nc.vector.tensor_mul(out=msq[:, :], in0=mu_pk[:, :],
                                         in1=mu_pk[:, :])
                    var = sqp.tile([128, fcol], F32, tag="pk_var", name="var")
                    nc.vector.tensor_sub(out=var[:, :], in0=m2_pk[:, :], in1=msq[:, :])
                    rstd_pk, var_eps, rstd_f = rsqrt_dve(
                        nc, sqp, var[:, :], "st_rs", eps=LN_EPS, out_dtype=BF16)
                    invr_pk = sqp.tile([128, fcol], BF16, tag="pk_invr", name="invr_pk")
                    nc.vector.tensor_mul(out=invr_pk[:, :], in0=var_eps[:, :],
                                         in1=rstd_f[:, :])
                    nmu_pk = sqp.tile([128, fcol], BF16, tag="pk_nmu", name="nmu_pk")
                    nc.vector.tensor_scalar_mul(out=nmu_pk[:, :], in0=mu_pk[:, :],
                                                scalar1=-1.0)
                    for nm, pk in (("negmu", nmu_pk), ("invr", invr_pk)):
                        nc.gpsimd.dma_start(
                            out=scr_stats[side][nm].ap().rearrange("(p f) -> p f", p=128),
                            in_=pk[:, :])
                for side in ("own", "oth"):
                    nc.gpsimd.dma_start(out=negmu[side][:, :],
                                        in_=scr_stats[side]["negmu"].ap()[None, :])
                    nc.gpsimd.dma_start(out=invr[side][:, :],
                                        in_=scr_stats[side]["invr"].ap()[None, :])

            # ---- P4+P5: projections + l2 norm, pipelined per head-pair ----
            # LayerNorm rstd cancels in the per-head l2 normalization, so q/k
            # are kept "raw" (rstd-unscaled); the k-side 1/|k| lands on the
            # partition axis of the transposed scores and is applied via the
            # exp's per-partition scale operand instead of scaling kT.
            invk_c16 = [persist.tile([128, 32], BF16, name=f"invk_c16{h}")
                        for h in range(H)]
            invk_col = [persist.tile([128, 32], F32, name=f"invk_col{h}")
                        for h in range(H)]
            invk_half = [persist.tile([128, 32], F32, name=f"invk_half{h}")
                         for h in range(H)]
            with tc.tile_pool(name="prps", bufs=2, space="PSUM") as prps, \
                 tc.tile_pool(name="l2", bufs=2) as l2p, \
                 tc.tile_pool(name="l2ps", bufs=2, space="PSUM") as l2ps:
                # v natural [s_oth, d] with ones column (no stats dependency)
                for sb in range(32):
                    ps = prps.tile([128, D], F32, tag="vps", name="vps")
                    ssl = slice(sb * 128, (sb + 1) * 128)
                    for c in range(4):
                        nc.tensor.matmul(ps[:, :], xT_oth[:, c, ssl], wvb[:, c, :],
                                         start=(c == 0), stop=False)
                    nc.tensor.matmul(ps[:, :], ones_row[:, :], bp_v[:, :],
                                     start=False, stop=True)
                    nc.scalar.copy(
                        out=vsb[:, sb // 2, sb % 2, :, 0:DH],
                        in_=ps[:, :].rearrange("p (h d) -> p h d", h=H))
                nc.vector.memset(vsb[:, :, :, :, DH:DH + 1], 1.0)

                def project_and_l2(oc, t, s, side, wb, csum, bp, scr_ssq,
                                   scr_inv, name):
                    osl = slice(oc * 128, (oc + 1) * 128)
                    for w in range(s // 512):
                        wsl = slice(w * 512, (w + 1) * 512)
                        ps = prps.tile([128, 512], F32, tag="pps", name="pps")
                        for c in range(4):
                            nc.tensor.matmul(ps[:, :], wb[:, c, osl],
                                             (xT_own if side == "own" else xT_oth)[:, c, wsl],
                                             start=(c == 0), stop=False)
                        nc.tensor.matmul(ps[:, :], csum[:, osl],
                                         negmu[side][:, wsl], start=False, stop=False)
                        nc.tensor.matmul(ps[:, :], bp[:, osl],
                                         invr[side][:, wsl], start=False, stop=True)
                        nc.vector.tensor_copy(out=t[:, oc, wsl], in_=ps[:, :])
                        sq = l2p.tile([128, 512], BF16, tag="l2sq", name="l2sq")
                        nc.scalar.activation(out=sq[:, :], in_=t[:, oc, wsl],
                                             func=AF.Square)
                        ssps = l2ps.tile([2, 512], F32, tag="l2ps", name="l2ps")
                        nc.tensor.matmul(ssps[:, :], hsel[:, :], sq[:, :],
                                         start=True, stop=True)
                        ssrow = l2p.tile([2, 512], F32, tag="ssrow", name="ssrow")
                        nc.vector.tensor_copy(out=ssrow[:, :], in_=ssps[:, :])
                        nc.gpsimd.dma_start(
                            out=bass.AP(tensor=scr_ssq,
                                        offset=2 * oc * s + w * 512,
                                        ap=[[s, 2], [1, 512]]),
                            in_=ssrow[:, :])
                    # packed inverse norms (contiguous reshape; rows preserved)
                    fcol = 2 * s // 128
                    pk = l2p.tile([128, fcol], F32, tag=f"l2pk_{name}", name="pk")
                    nc.gpsimd.dma_start(
                        out=pk[:, :],
                        in_=bass.AP(tensor=scr_ssq, offset=2 * oc * s,
                                    ap=[[fcol, 128], [1, fcol]]))
                    ipk, _, _ = rsqrt_dve(
                        nc, l2p, pk[:, :], f"l2rs_{name}", eps=L2_EPS2,
                        out_dtype=BF16,
                        post_scale=(0.125 if name == "q" else None))
                    nc.gpsimd.dma_start(
                        out=bass.AP(tensor=scr_inv, offset=2 * oc * s,
                                    ap=[[fcol, 128], [1, fcol]]),
                        in_=ipk[:, :])
                    if name == "q":
                        rep = l2p.tile([128, S_OWN], BF16, name="l2rep",
                                       tag="l2rep")
                        nc.gpsimd.dma_start(
                            out=rep[0:64, :],
                            in_=bcast_ap(scr_inv, (2 * oc) * s, 64, s))
                        nc.gpsimd.dma_start(
                            out=rep[64:128, :],
                            in_=bcast_ap(scr_inv, (2 * oc + 1) * s, 64, s))
                        nc.vector.tensor_mul(out=t[:, oc, :], in0=t[:, oc, :],
                                             in1=rep[:, :])
                    else:
                        for j in range(2):
                            h = 2 * oc + j
                            nc.sync.dma_start_transpose(
                                out=invk_c16[h][:, :],
                                in_=bass.AP(tensor=scr_inv, offset=h * s,
                                            ap=[[128, 32], [1, 128]]))
                            nc.vector.tensor_copy(out=invk_col[h][:, :],
                                                  in_=invk_c16[h][:, :])
                            nc.vector.tensor_scalar_mul(out=invk_half[h][:, :],
                                                        in0=invk_col[h][:, :],
                                                        scalar1=0.5)

                for oc in range(4):
                    project_and_l2(oc, qT, S_OWN, "own", wqb, csum_q, bp_q,
                                   scr_ssq_q, scr_inv_q, "q")
                    project_and_l2(oc, kT, S_OTH, "oth", wkb, csum_k, bp_k,
                                   scr_ssq_k, scr_inv_k, "k")

            xT_oth_cm.__exit__(None, None, None)

            # ================= P6: attention =================
            # head pairs; full-width scores [128, 1024]; the partner head's
            # matmuls hide the exp latency so PE never stalls on ACT.
            with tc.tile_pool(name="scps", bufs=1, space="PSUM") as scps, \
                 tc.tile_pool(name="ctps", bufs=1, space="PSUM") as ctps, \
                 tc.tile_pool(name="att", bufs=3) as attp, \
                 tc.tile_pool(name="attr", bufs=2) as attrp:
                for hp in range(4):
                    ctx2 = [ctps.tile([DH + 1, S_OWN], F32, name=f"ctx{j}",
                                      tag=f"ctx{j}") for j in range(2)]
                    for kcp in range(16):
                        e2 = [attp.tile([128, 2, S_OWN], mybir.dt.float8e4,
                                        name=f"e{j}", tag=f"e{j}")
                              for j in range(2)]
                        for i in range(2):
                            kc = 2 * kcp + i
                            ksl = slice(kc * 128, (kc + 1) * 128)
                            for j in range(2):
                                psl = slice(64 * j, 64 * (j + 1))
                                sc = scps.tile([128, S_OWN], F32, name=f"sc{j}",
                                               tag=f"sc{j}")
                                nc.tensor.matmul(sc[:, 0:512], kT[psl, hp, ksl],
                                                 qT[psl, hp, 0:512],
                                                 start=True, stop=True)
                                nc.tensor.matmul(sc[:, 512:1024], kT[psl, hp, ksl],
                                                 qT[psl, hp, 512:1024],
                                                 start=True, stop=True)
                                h = 2 * hp + j
                                if i == 0 and j == 0 and kcp % 8 < 5:
                                    # exp(s) ~= (1 + s/2)^2 on DVE (|s| <= 1/8)
                                    u = attp.tile([128, S_OWN], F32, name="expu",
                                                  tag="expu")
                                    nc.vector.tensor_scalar(
                                        out=u[:, :], in0=sc[:, :],
                                        scalar1=invk_half[h][:, kc:kc + 1],
                                        scalar2=1.0,
                                        op0=ALU.mult, op1=ALU.add)
                                    nc.vector.tensor_mul(out=e2[j][:, i, :],
                                                         in0=u[:, :], in1=u[:, :])
                                else:
                                    nc.scalar.activation(
                                        out=e2[j][:, i, :], in_=sc[:, :],
                                        func=AF.Exp,
                                        scale=invk_col[h][:, kc:kc + 1])
                        for j in range(2):
                            nc.tensor.matmul(
                                ctx2[j][:, 0:512],
                                vsb[:, kcp, :, 2 * hp + j, 0:DH + 1],
                                e2[j][:, :, 0:512],
                                start=(kcp == 0), stop=(kcp == 15),
                                perf_mode=mybir.MatmulPerfMode.DoubleRow)
                            nc.tensor.matmul(
                                ctx2[j][:, 512:1024],
                                vsb[:, kcp, :, 2 * hp + j, 0:DH + 1],
                                e2[j][:, :, 512:1024],
                                start=(kcp == 0), stop=(kcp == 15),
                                perf_mode=mybir.MatmulPerfMode.DoubleRow)
                    for j in range(2):
                        h = 2 * hp + j
                        psl = slice(64 * j, 64 * (j + 1))
                        rs = attrp.tile([DH + 1, S_OWN], F32, tag="rs")
                        nc.vector.reciprocal(out=rs[DH:DH + 1, :],
                                             in_=ctx2[j][DH:DH + 1, :])
                        rs16 = attrp.tile([DH + 1, S_OWN], BF16, tag="rs16")
                        nc.vector.tensor_copy(out=rs16[DH:DH + 1, :],
                                              in_=rs[DH:DH + 1, :])
                        nc.gpsimd.dma_start(out=scr_rsum.ap()[h:h + 1, :],
                                            in_=rs16[DH:DH + 1, :])
                        rep = attrp.tile([64, S_OWN], BF16, tag="rsrep")
                        nc.gpsimd.dma_start(
                            out=rep[:, :],
                            in_=bcast_ap(scr_rsum, h * S_OWN, 64, S_OWN))
                        nc.vector.tensor_mul(out=ctxT[psl, hp, :],
                                             in0=ctx2[j][0:DH, :], in1=rep[:, :])

            # ================= P7: output proj + gate + residual =================
            with tc.tile_pool(name="ops", bufs=2, space="PSUM") as opsp, \
                 tc.tile_pool(name="fin", bufs=1) as finp, \
                 tc.tile_pool(name="fin3", bufs=3) as fin3:
                zs, projs = [], []
                mv_all = finp.tile([128, 2, 8], F32)
                for sb in range(8):
                    ssl = slice(sb * 128, (sb + 1) * 128)
                    ps_o = opsp.tile([128, D], F32, tag="pso", name="pso")
                    for c in range(4):
                        nc.tensor.matmul(ps_o[:, :], ctxT[:, c, ssl], wob[:, c, :],
                                         start=(c == 0), stop=False)
                    nc.tensor.matmul(ps_o[:, :], ones_row[:, :], bo_row[:, :],
                                     start=False, stop=True)
                    proj = finp.tile([128, D], BF16, tag=f"proj{sb}",
                                     name=f"proj{sb}")
                    nc.scalar.copy(out=proj[:, :], in_=ps_o[:, :])
                    projs.append(proj)

                    ps_z = opsp.tile([128, D], F32, tag="psz", name="psz")
                    for c in range(4):
                        nc.tensor.matmul(ps_z[:, :], ctxT[:, c, ssl], wgb[:, c, :],
                                         start=(c == 0), stop=False)
                    for c in range(4):
                        nc.tensor.matmul(ps_z[:, :], xT_own[:, c, ssl],
                                         wgb[:, 4 + c, :], start=False, stop=False)
                    nc.tensor.matmul(ps_z[:, :], ones_row[:, :], bg_row[:, :],
                                     start=False, stop=True)
                    z = finp.tile([128, D], BF16, tag=f"z{sb}", name=f"z{sb}")
                    nc.scalar.copy(out=z[:, :], in_=ps_z[:, :])
                    zs.append(z)
                    stats = fin3.tile([128, 6], F32, tag="st6", name="st6")
                    nc.vector.bn_stats(out=stats[:, :], in_=z[:, :])
                    nc.vector.bn_aggr(out=mv_all[:, :, sb], in_=stats[:, :])

                rstd_all, _, _ = rsqrt_dve(nc, finp, mv_all[:, 1, :], "g_rs",
                                           eps=LN_EPS)
                for sb in range(8):
                    ssl = slice(sb * 128, (sb + 1) * 128)
                    z, proj = zs[sb], projs[sb]
                    zn = fin3.tile([128, D], F32, tag="zn", name="zn")
                    nc.vector.tensor_scalar(out=zn[:, :], in0=z[:, :],
                                            scalar1=mv_all[:, 0:1, sb],
                                            scalar2=rstd_all[:, sb:sb + 1],
                                            op0=ALU.subtract, op1=ALU.mult)
                    zg = fin3.tile([128, D], F32, tag="zg", name="zg")
                    nc.vector.tensor_mul(out=zg[:, :], in0=zn[:, :], in1=gg_rep[:, :])
                    nc.vector.tensor_add(out=zg[:, :], in0=zg[:, :], in1=gb_rep[:, :])
                    gate = fin3.tile([128, D], F32, tag="gate", name="gate")
                    nc.scalar.activation(out=gate[:, :], in_=zg[:, :],
                                         func=AF.Sigmoid)
                    xblk = fin3.tile([128, D], F32, tag="xblk", name="xblk")
                    nc.sync.dma_start(out=xblk[:, :], in_=xf_own.ap()[ssl, :])
                    gp = fin3.tile([128, D], F32, tag="gp", name="gp")
                    nc.vector.tensor_mul(out=gp[:, :], in0=gate[:, :],
                                         in1=proj[:, :])
                    ob = fin3.tile([128, D], F32, tag="ob", name="ob")
                    nc.vector.tensor_add(out=ob[:, :], in0=gp[:, :], in1=xblk[:, :])
                    nc.sync.dma_start(out=out_d.ap()[ssl, :], in_=ob[:, :])

    nc.compile()
    return nc


_NC_CACHE = None


def _get_nc():
    global _NC_CACHE
    if _NC_CACHE is None:
        _NC_CACHE = build_nc()
    return _NC_CACHE


def make_in_maps(inputs):
    xg = np.ascontiguousarray(np.asarray(inputs["gene_embeds"], np.float32))
    xd = np.ascontiguousarray(np.asarray(inputs["drug_embeds"], np.float32))
    xg16 = xg.astype(ml_dtypes.bfloat16)
    xd16 = xd.astype(ml_dtypes.bfloat16)

    f32 = lambda k: np.ascontiguousarray(np.asarray(inputs[k], np.float32))

    gene_common = dict(
        x16_oth=xd16, wq=f32("wgq"), wk=f32("wdk"), wv=f32("wdv"), wo=f32("wo"),
        wg=f32("wgg"), bq=f32("bgq"), bk=f32("bdk"), bv=f32("bdv"), bo=f32("bo"),
        bg=f32("bgg"), g_own=f32("lng_g"), b_own=f32("lng_b"), g_oth=f32("lnd_g"),
        b_oth=f32("lnd_b"), gg=f32("gg_g"), gb=f32("gg_b"))
    drug_common = dict(
        x16_oth=xg16, wq=f32("wdq"), wk=f32("wgk"), wv=f32("wgv"), wo=f32("wo"),
        wg=f32("wdg"), bq=f32("bdq"), bk=f32("bgk"), bv=f32("bgv"), bo=f32("bo"),
        bg=f32("bdg"), g_own=f32("lnd_g"), b_own=f32("lnd_b"), g_oth=f32("lng_g"),
        b_oth=f32("lng_b"), gg=f32("dg_g"), gb=f32("dg_b"))

    in_maps = []
    for i in range(8):
        if i < 4:
            sl = slice(i * S_OWN, (i + 1) * S_OWN)
            m = dict(gene_common)
            m["x16_own"] = np.ascontiguousarray(xg16[sl])
            m["xf_own"] = np.ascontiguousarray(xg[sl])
        else:
            sl = slice((i - 4) * S_OWN, (i - 3) * S_OWN)
            m = dict(drug_common)
            m["x16_own"] = np.ascontiguousarray(xd16[sl])
            m["xf_own"] = np.ascontiguousarray(xd[sl])
        in_maps.append(m)
    return in_maps


def kernel(**inputs):
    nc = _get_nc()
    in_maps = make_in_maps(inputs)
    res = run_bass_kernel_spmd(nc, in_maps, core_ids=list(range(8)))
    gene_out = np.concatenate([res.results[i]["out"] for i in range(4)], axis=0)
    drug_out = np.concatenate([res.results[i]["out"] for i in range(4, 8)], axis=0)
    return (gene_out, drug_out)


# revision 31
# speedup vs baseline: 4458.5289x; 1.0318x over previous
"""Trainium2 Bass kernel for EnhancedCrossAttention (dense transformer, 8-core SPMD).

Sharding: cores 0-3 compute gene_out rows [1024*i, 1024*(i+1)) attending over all
drug K/V; cores 4-7 mirror for drug_out. One SPMD program with direction-generic
input names; host slices/replicates inputs and concatenates outputs.

Layout strategy: all activations live transposed [dims, seq] on-chip (loaded via
DMA-xbar transpose); LayerNorm is folded into the projection matmuls as rank-1
PSUM corrections, so q/k/v projections consume the raw transposed embeds
directly and produce qT/kT ready for the score matmuls with no PE transposes.
Scores are computed transposed [k, q] so the exp'd tile is directly the lhsT of
the context matmul; a ones-column in V yields softmax denominators for free.
"""
import numpy as np
import ml_dtypes

import concourse.bass as bass
import concourse.mybir as mybir
import concourse.tile as tile
from concourse import bacc
from concourse.bass_utils import run_bass_kernel_spmd

F32 = mybir.dt.float32
BF16 = mybir.dt.bfloat16
AF = mybir.ActivationFunctionType
ALU = mybir.AluOpType

D = 512
H = 8
DH = 64
S_OWN = 1024   # query rows per core
S_OTH = 4096   # key/value rows (full opposite side)
NC = 8

LN_EPS = 1e-5
L2_EPS2 = 1e-24          # eps^2 for l2 norm (ref: max(norm, 1e-12))
QSCALE_LOG = float(np.log(0.125))  # DH ** -0.5 folded into inv-norm of q


def build_nc():
    nc = bacc.Bacc("TRN2", target_bir_lowering=False, debug=False, num_devices=NC)

    # ---- DRAM I/O ----
    x16_own = nc.dram_tensor("x16_own", [S_OWN, D], BF16, kind="ExternalInput")
    x16_oth = nc.dram_tensor("x16_oth", [S_OTH, D], BF16, kind="ExternalInput")
    xf_own = nc.dram_tensor("xf_own", [S_OWN, D], F32, kind="ExternalInput")
    wq_d = nc.dram_tensor("wq", [D, D], F32, kind="ExternalInput")
    wk_d = nc.dram_tensor("wk", [D, D], F32, kind="ExternalInput")
    wv_d = nc.dram_tensor("wv", [D, D], F32, kind="ExternalInput")
    wo_d = nc.dram_tensor("wo", [D, D], F32, kind="ExternalInput")
    wg_d = nc.dram_tensor("wg", [2 * D, D], F32, kind="ExternalInput")
    bq_d = nc.dram_tensor("bq", [D], F32, kind="ExternalInput")
    bk_d = nc.dram_tensor("bk", [D], F32, kind="ExternalInput")
    bv_d = nc.dram_tensor("bv", [D], F32, kind="ExternalInput")
    bo_d = nc.dram_tensor("bo", [D], F32, kind="ExternalInput")
    bg_d = nc.dram_tensor("bg", [D], F32, kind="ExternalInput")
    g_own_d = nc.dram_tensor("g_own", [D], F32, kind="ExternalInput")
    b_own_d = nc.dram_tensor("b_own", [D], F32, kind="ExternalInput")
    g_oth_d = nc.dram_tensor("g_oth", [D], F32, kind="ExternalInput")
    b_oth_d = nc.dram_tensor("b_oth", [D], F32, kind="ExternalInput")
    gg_d = nc.dram_tensor("gg", [D], F32, kind="ExternalInput")
    gb_d = nc.dram_tensor("gb", [D], F32, kind="ExternalInput")
    out_d = nc.dram_tensor("out", [S_OWN, D], F32, kind="ExternalOutput")

    # DRAM scratch for row replication round-trips
    scr_stats = {}
    for side, s in (("oth", S_OTH), ("own", S_OWN)):
        scr_stats[side] = {
            "mu": nc.dram_tensor(f"scr_mu_{side}", [s], F32),
            "m2": nc.dram_tensor(f"scr_m2_{side}", [s], F32),
            "negmu": nc.dram_tensor(f"scr_negmu_{side}", [s], BF16),
            "rstd": nc.dram_tensor(f"scr_rstd_{side}", [s], BF16),
            "invr": nc.dram_tensor(f"scr_invr_{side}", [s], BF16),
        }
    scr_ssq_q = nc.dram_tensor("scr_ssq_q", [H, S_OWN], F32)
    scr_ssq_k = nc.dram_tensor("scr_ssq_k", [H, S_OTH], F32)
    scr_inv_q = nc.dram_tensor("scr_inv_q", [H, S_OWN], BF16)
    scr_inv_k = nc.dram_tensor("scr_inv_k", [H, S_OTH], BF16)
    scr_rsum = nc.dram_tensor("scr_rsum", [H, S_OWN], BF16)

    def bcast_ap(dram, offset, nrep, n):
        return bass.AP(tensor=dram, offset=offset, ap=[[0, nrep], [1, n]])

    I32 = mybir.dt.int32
    MAGIC = 0x5F3759DF

    def rsqrt_dve(nc, pool, x, tag, eps=0.0, newton=2, out_dtype=F32,
                  post_scale=None):
        """out = post_scale * 1/sqrt(x + eps), all on DVE (no ACT tables)."""
        p, f = x.shape[0], x.free_size()
        xe = pool.tile([p, f], F32, name=f"{tag}_xe", tag=f"{tag}_xe")
        if eps:
            nc.vector.tensor_scalar_add(out=xe[:, :], in0=x, scalar1=float(eps))
        else:
            nc.vector.tensor_copy(out=xe[:, :], in_=x)
        it = pool.tile([p, f], I32, name=f"{tag}_it", tag=f"{tag}_it")
        nc.vector.tensor_scalar(out=it[:, :], in0=xe[:, :].bitcast(I32),
                                scalar1=1, scalar2=None,
                                op0=ALU.arith_shift_right)
        nc.vector.tensor_scalar(out=it[:, :], in0=it[:, :],
                                scalar1=-1, scalar2=MAGIC,
                                op0=ALU.mult, op1=ALU.add)
        y = pool.tile([p, f], F32, name=f"{tag}_y", tag=f"{tag}_y")
        nc.vector.tensor_copy(out=y[:, :], in_=it[:, :].bitcast(F32))
        t1 = pool.tile([p, f], F32, name=f"{tag}_t1", tag=f"{tag}_t1")
        for _ in range(newton):
            nc.vector.tensor_mul(out=t1[:, :], in0=y[:, :], in1=y[:, :])
            nc.vector.tensor_mul(out=t1[:, :], in0=t1[:, :], in1=xe[:, :])
            nc.vector.tensor_scalar(out=t1[:, :], in0=t1[:, :],
                                    scalar1=-0.5, scalar2=1.5,
                                    op0=ALU.mult, op1=ALU.add)
            nc.vector.tensor_mul(out=y[:, :], in0=y[:, :], in1=t1[:, :])
        out = pool.tile([p, f], out_dtype, name=f"{tag}_o", tag=f"{tag}_o")
        if post_scale is not None:
            nc.vector.tensor_scalar_mul(out=out[:, :], in0=y[:, :],
                                        scalar1=float(post_scale))
        else:
            nc.vector.tensor_copy(out=out[:, :], in_=y[:, :])
        return out, xe, y

    with tile.TileContext(nc) as tc:
        with tc.tile_pool(name="persist", bufs=1) as persist:
            # ---- constants ----
            ones_col = persist.tile([128, 1], BF16)       # K=128->M=1 colsum
            nc.vector.memset(ones_col, 1.0)
            oD_col = persist.tile([128, 1], BF16)         # 1/D for mean
            nc.vector.memset(oD_col, 1.0 / D)
            ones_row = persist.tile([1, 128], BF16)       # K=1 lhsT for bias outer
            nc.vector.memset(ones_row, 1.0)
            hsel = persist.tile([128, 2], BF16)           # per-head-pair colsum
            nc.vector.memset(hsel, 0.0)
            nc.vector.memset(hsel[0:64, 0:1], 1.0)
            nc.vector.memset(hsel[64:128, 1:2], 1.0)
            eps_col = persist.tile([128, 1], F32)
            nc.vector.memset(eps_col, LN_EPS)
            eps24_col = persist.tile([128, 1], F32)
            nc.vector.memset(eps24_col, L2_EPS2)
            qlog_col = persist.tile([128, 1], F32)
            nc.vector.memset(qlog_col, QSCALE_LOG)
            zero_col = persist.tile([128, 1], F32)
            nc.vector.memset(zero_col, 0.0)

            # ---- persistent SBUF tensors ----
            xT_own = persist.tile([128, 4, S_OWN], BF16)
            qT = persist.tile([128, 4, S_OWN], BF16)
            kT = persist.tile([128, 4, S_OTH], BF16)
            vsb = persist.tile([128, 16, 2, H, 72], mybir.dt.float8e4)
            ctxT = persist.tile([128, 4, S_OWN], BF16)
            wqb = persist.tile([128, 4, D], BF16)
            wkb = persist.tile([128, 4, D], BF16)
            wvb = persist.tile([128, 4, D], BF16)
            wob = persist.tile([128, 4, D], BF16)
            wgb = persist.tile([128, 8, D], BF16)
            csum_q = persist.tile([1, D], BF16)
            csum_k = persist.tile([1, D], BF16)
            bp_q = persist.tile([1, D], BF16)
            bp_k = persist.tile([1, D], BF16)
            bp_v = persist.tile([1, D], BF16)
            bo_row = persist.tile([1, D], BF16)
            bg_row = persist.tile([1, D], BF16)
            gg_rep = persist.tile([128, D], F32)
            gb_rep = persist.tile([128, D], F32)
            negmu = {"own": persist.tile([1, S_OWN], BF16, name="negmu_own"),
                     "oth": persist.tile([1, S_OTH], BF16, name="negmu_oth")}
            invr = {"own": persist.tile([1, S_OWN], BF16, name="invr_own"),
                    "oth": persist.tile([1, S_OTH], BF16, name="invr_oth")}

            # ================= P1: weight prep =================
            with tc.tile_pool(name="wstage", bufs=2) as wstage, \
                 tc.tile_pool(name="wpsum", bufs=2, space="PSUM") as wpsum:
                gcols = {}
                for nm, dram in (("g_own", g_own_d), ("b_own", b_own_d),
                                 ("g_oth", g_oth_d), ("b_oth", b_oth_d)):
                    t = wstage.tile([128, 4], F32, tag=f"gcol_{nm}")
                    nc.sync.dma_start(out=t[:, :],
                                      in_=dram.ap().rearrange("(c p) -> p c", p=128))
                    gcols[nm] = t

                def prep_qk(w_dram, b_dram, g_nm, bln_nm, wb, csum, bp):
                    wst = wstage.tile([128, 4, D], F32, tag="wst")
                    nc.sync.dma_start(out=wst[:, :, :],
                                      in_=w_dram.ap().rearrange("(c p) d -> p c d", p=128))
                    for c in range(4):
                        nc.vector.tensor_scalar_mul(out=wb[:, c, :], in0=wst[:, c, :],
                                                    scalar1=gcols[g_nm][:, c:c + 1])
                    ps = wpsum.tile([1, D], F32, tag="wps")
                    for c in range(4):
                        nc.tensor.matmul(ps[:, :], ones_col[:, :], wb[:, c, :],
                                         start=(c == 0), stop=(c == 3))
                    nc.vector.tensor_copy(out=csum[:, :], in_=ps[:, :])
                    ps2 = wpsum.tile([1, D], F32, tag="wps")
                    for c in range(4):
                        nc.tensor.matmul(ps2[:, :], gcols[bln_nm][:, c:c + 1],
                                         wst[:, c, :], start=(c == 0), stop=(c == 3))
                    brow = wstage.tile([1, D], F32, tag="brow")
                    nc.sync.dma_start(out=brow[:, :], in_=b_dram.ap()[None, :])
                    bsum = wstage.tile([1, D], F32, tag="bsum")
                    nc.vector.tensor_add(out=bsum[:, :], in0=ps2[:, :], in1=brow[:, :])
                    nc.vector.tensor_copy(out=bp[:, :], in_=bsum[:, :])

                prep_qk(wq_d, bq_d, "g_own", "b_own", wqb, csum_q, bp_q)
                prep_qk(wk_d, bk_d, "g_oth", "b_oth", wkb, csum_k, bp_k)

                for w_dram, wb in ((wv_d, wvb), (wo_d, wob)):
                    wst = wstage.tile([128, 4, D], F32, tag="wst")
                    nc.sync.dma_start(out=wst[:, :, :],
                                      in_=w_dram.ap().rearrange("(c p) d -> p c d", p=128))
                    for c in range(4):
                        nc.vector.tensor_copy(out=wb[:, c, :], in_=wst[:, c, :])
                wst8 = wstage.tile([128, 8, D], F32, tag="wst8")
                nc.sync.dma_start(out=wst8[:, :, :],
                                  in_=wg_d.ap().rearrange("(c p) d -> p c d", p=128))
                for c in range(8):
                    nc.vector.tensor_copy(out=wgb[:, c, :], in_=wst8[:, c, :])

                for b_dram, row in ((bv_d, bp_v), (bo_d, bo_row), (bg_d, bg_row)):
                    br = wstage.tile([1, D], F32, tag="brow")
                    nc.sync.dma_start(out=br[:, :], in_=b_dram.ap()[None, :])
                    nc.vector.tensor_copy(out=row[:, :], in_=br[:, :])

                for dram, rep in ((gg_d, gg_rep), (gb_d, gb_rep)):
                    nc.sync.dma_start(out=rep[:, :], in_=bcast_ap(dram, 0, 128, D))

            # ================= P2-P5 (need xT_oth alive) =================
            xT_oth_cm = tc.tile_pool(name="xT_oth", bufs=1)
            xT_oth_pool = xT_oth_cm.__enter__()
            xT_oth = xT_oth_pool.tile([128, 4, S_OTH], BF16)
            for c in range(4):
                nc.sync.dma_start_transpose(out=xT_own[:, c, :],
                                            in_=x16_own.ap()[:, c * 128:(c + 1) * 128])
                nc.sync.dma_start_transpose(out=xT_oth[:, c, :],
                                            in_=x16_oth.ap()[:, c * 128:(c + 1) * 128])

            # ---- P3: LN stats ----
            with tc.tile_pool(name="sq", bufs=2) as sqp, \
                 tc.tile_pool(name="stps", bufs=2, space="PSUM") as stps:
                for side, s, xt in (("oth", S_OTH, xT_oth), ("own", S_OWN, xT_own)):
                    nwin = s // 512
                    for w in range(nwin):
                        wsl = slice(w * 512, (w + 1) * 512)
                        ps_mu = stps.tile([1, 512], F32, tag="psmu", name="psmu")
                        ps_m2 = stps.tile([1, 512], F32, tag="psm2", name="psm2")
                        for c in range(4):
                            sq = sqp.tile([128, 512], BF16, tag="sq", name="sq")
                            nc.scalar.activation(out=sq[:, :], in_=xt[:, c, wsl],
                                                 func=AF.Square)
                            nc.tensor.matmul(ps_mu[:, :], oD_col[:, :], xt[:, c, wsl],
                                             start=(c == 0), stop=(c == 3))
                            nc.tensor.matmul(ps_m2[:, :], oD_col[:, :], sq[:, :],
                                             start=(c == 0), stop=(c == 3))
                        strow_mu = sqp.tile([1, 512], F32, tag="strow_mu",
                                            name="strow_mu")
                        strow_m2 = sqp.tile([1, 512], F32, tag="strow_m2",
                                            name="strow_m2")
                        nc.vector.tensor_copy(out=strow_mu[:, :], in_=ps_mu[:, :])
                        nc.vector.tensor_copy(out=strow_m2[:, :], in_=ps_m2[:, :])
                        nc.gpsimd.dma_start(
                            out=scr_stats[side]["mu"].ap()[wsl][None, :],
                            in_=strow_mu[:, :])
                        nc.gpsimd.dma_start(
                            out=scr_stats[side]["m2"].ap()[wsl][None, :],
                            in_=strow_m2[:, :])
                for side, s in (("oth", S_OTH), ("own", S_OWN)):
                    fcol = s // 128
                    mu_pk = sqp.tile([128, fcol], F32, tag="pk_mu", name="mu_pk")
                    m2_pk = sqp.tile([128, fcol], F32, tag="pk_m2", name="m2_pk")
                    nc.gpsimd.dma_start(
                        out=mu_pk[:, :],
                        in_=scr_stats[side]["mu"].ap().rearrange("(p f) -> p f", p=128))
                    nc.gpsimd.dma_start(
                        out=m2_pk[:, :],
                        in_=scr_stats[side]["m2"].ap().rearrange("(p f) -> p f", p=128))
                    msq = sqp.tile([128, fcol], F32, tag="pk_msq", name="msq")
                    nc.vector.tensor_mul(out=msq[:, :], in0=mu_pk[:, :],
                                         in1=mu_pk[:, :])
                    var = sqp.tile([128, fcol], F32, tag="pk_var", name="var")
                    nc.vector.tensor_sub(out=var[:, :], in0=m2_pk[:, :], in1=msq[:, :])
                    rstd_pk, var_eps, rstd_f = rsqrt_dve(
                        nc, sqp, var[:, :], "st_rs", eps=LN_EPS, out_dtype=BF16)
                    invr_pk = sqp.tile([128, fcol], BF16, tag="pk_invr", name="invr_pk")
                    nc.vector.tensor_mul(out=invr_pk[:, :], in0=var_eps[:, :],
                                         in1=rstd_f[:, :])
                    nmu_pk = sqp.tile([128, fcol], BF16, tag="pk_nmu", name="nmu_pk")
                    nc.vector.tensor_scalar_mul(out=nmu_pk[:, :], in0=mu_pk[:, :],
                                                scalar1=-1.0)
                    for nm, pk in (("negmu", nmu_pk), ("invr", invr_pk)):
                        nc.gpsimd.dma_start(
                            out=scr_stats[side][nm].ap().rearrange("(p f) -> p f", p=128),
                            in_=pk[:, :])
                for side in ("own", "oth"):
                    nc.gpsimd.dma_start(out=negmu[side][:, :],
                                        in_=scr_stats[side]["negmu"].ap()[None, :])
                    nc.gpsimd.dma_start(out=invr[side][:, :],
                                        in_=scr_stats[side]["invr"].ap()[None, :])

            # ---- P4+P5: projections + l2 norm, pipelined per head-pair ----
            # LayerNorm rstd cancels in the per-head l2 normalization, so q/k
            # are kept "raw" (rstd-unscaled); the k-side 1/|k| lands on the
            # partition axis of the transposed scores and is applied via the
            # exp's per-partition scale operand instead of scaling kT.
            invk_c16 = [persist.tile([128, 32], BF16, name=f"invk_c16{h}")
                        for h in range(H)]
            invk_col = [persist.tile([128, 32], F32, name=f"invk_col{h}")
                        for h in range(H)]
            invk_half = [persist.tile([128, 32], F32, name=f"invk_half{h}")
                         for h in range(H)]
            with tc.tile_pool(name="prps", bufs=2, space="PSUM") as prps, \
                 tc.tile_pool(name="l2", bufs=2) as l2p, \
                 tc.tile_pool(name="l2ps", bufs=2, space="PSUM") as l2ps:
                # v natural [s_oth, d] with ones column (no stats dependency)
                for sb in range(32):
                    ps = prps.tile([128, D], F32, tag="vps", name="vps")
                    ssl = slice(sb * 128, (sb + 1) * 128)
                    for c in range(4):
                        nc.tensor.matmul(ps[:, :], xT_oth[:, c, ssl], wvb[:, c, :],
                                         start=(c == 0), stop=False)
                    nc.tensor.matmul(ps[:, :], ones_row[:, :], bp_v[:, :],
                                     start=False, stop=True)
                    nc.scalar.copy(
                        out=vsb[:, sb // 2, sb % 2, :, 0:DH],
                        in_=ps[:, :].rearrange("p (h d) -> p h d", h=H))
                nc.vector.memset(vsb[:, :, :, :, DH:DH + 1], 1.0)

                def project_and_l2(oc, t, s, side, wb, csum, bp, scr_ssq,
                                   scr_inv, name):
                    osl = slice(oc * 128, (oc + 1) * 128)
                    for w in range(s // 512):
                        wsl = slice(w * 512, (w + 1) * 512)
                        ps = prps.tile([128, 512], F32, tag="pps", name="pps")
                        for c in range(4):
                            nc.tensor.matmul(ps[:, :], wb[:, c, osl],
                                             (xT_own if side == "own" else xT_oth)[:, c, wsl],
                                             start=(c == 0), stop=False)
                        nc.tensor.matmul(ps[:, :], csum[:, osl],
                                         negmu[side][:, wsl], start=False, stop=False)
                        nc.tensor.matmul(ps[:, :], bp[:, osl],
                                         invr[side][:, wsl], start=False, stop=True)
                        nc.vector.tensor_copy(out=t[:, oc, wsl], in_=ps[:, :])
                        sq = l2p.tile([128, 512], BF16, tag="l2sq", name="l2sq")
                        nc.scalar.activation(out=sq[:, :], in_=t[:, oc, wsl],
                                             func=AF.Square)
                        ssps = l2ps.tile([2, 512], F32, tag="l2ps", name="l2ps")
                        nc.tensor.matmul(ssps[:, :], hsel[:, :], sq[:, :],
                                         start=True, stop=True)
                        ssrow = l2p.tile([2, 512], F32, tag="ssrow", name="ssrow")
                        nc.vector.tensor_copy(out=ssrow[:, :], in_=ssps[:, :])
                        nc.gpsimd.dma_start(
                            out=bass.AP(tensor=scr_ssq,
                                        offset=2 * oc * s + w * 512,
                                        ap=[[s, 2], [1, 512]]),
                            in_=ssrow[:, :])
                    # packed inverse norms (contiguous reshape; rows preserved)
                    fcol = 2 * s // 128
                    pk = l2p.tile([128, fcol], F32, tag=f"l2pk_{name}", name="pk")
                    nc.gpsimd.dma_start(
                        out=pk[:, :],
                        in_=bass.AP(tensor=scr_ssq, offset=2 * oc * s,
                                    ap=[[fcol, 128], [1, fcol]]))
                    ipk, _, _ = rsqrt_dve(
                        nc, l2p, pk[:, :], f"l2rs_{name}", eps=L2_EPS2,
                        out_dtype=BF16,
                        post_scale=(0.125 if name == "q" else None))
                    nc.gpsimd.dma_start(
                        out=bass.AP(tensor=scr_inv, offset=2 * oc * s,
                                    ap=[[fcol, 128], [1, fcol]]),
                        in_=ipk[:, :])
                    if name == "q":
                        rep = l2p.tile([128, S_OWN], BF16, name="l2rep",
                                       tag="l2rep")
                        nc.gpsimd.dma_start(
                            out=rep[0:64, :],
                            in_=bcast_ap(scr_inv, (2 * oc) * s, 64, s))
                        nc.gpsimd.dma_start(
                            out=rep[64:128, :],
                            in_=bcast_ap(scr_inv, (2 * oc + 1) * s, 64, s))
                        nc.vector.tensor_mul(out=t[:, oc, :], in0=t[:, oc, :],
                                             in1=rep[:, :])
                    else:
                        for j in range(2):
                            h = 2 * oc + j
                            nc.sync.dma_start_transpose(
                                out=invk_c16[h][:, :],
                                in_=bass.AP(tensor=scr_inv, offset=h * s,
                                            ap=[[128, 32], [1, 128]]))
                            nc.vector.tensor_copy(out=invk_col[h][:, :],
                                                  in_=invk_c16[h][:, :])
                            nc.vector.tensor_scalar_mul(out=invk_half[h][:, :],
                                                        in0=invk_col[h][:, :],
                                                        scalar1=0.5)

                for oc in range(4):
                    project_and_l2(oc, qT, S_OWN, "own", wqb, csum_q, bp_q,
                                   scr_ssq_q, scr_inv_q, "q")
                    project_and_l2(oc, kT, S_OTH, "oth", wkb, csum_k, bp_k,
                                   scr_ssq_k, scr_inv_k, "k")

            xT_oth_cm.__exit__(None, None, None)

            # ================= P6: attention =================
            # head pairs; full-width scores [128, 1024]; the partner head's
            # matmuls hide the exp latency so PE never stalls on ACT.
            with tc.tile_pool(name="scps", bufs=1, space="PSUM") as scps, \
                 tc.tile_pool(name="ctps", bufs=1, space="PSUM") as ctps, \
                 tc.tile_pool(name="att", bufs=3) as attp, \
                 tc.tile_pool(name="attr", bufs=2) as attrp:
                for hp in range(4):
                    ctx2 = [ctps.tile([DH + 1, S_OWN], F32, name=f"ctx{j}",
                                      tag=f"ctx{j}") for j in range(2)]
                    for kcp in range(16):
                        e2 = [attp.tile([128, 2, S_OWN], mybir.dt.float8e4,
                                        name=f"e{j}", tag=f"e{j}")
                              for j in range(2)]
                        for i in range(2):
                            kc = 2 * kcp + i
                            ksl = slice(kc * 128, (kc + 1) * 128)
                            for j in range(2):
                                psl = slice(64 * j, 64 * (j + 1))
                                sc = scps.tile([128, S_OWN], F32, name=f"sc{j}",
                                               tag=f"sc{j}")
                                nc.tensor.matmul(sc[:, 0:512], kT[psl, hp, ksl],
                                                 qT[psl, hp, 0:512],
                                                 start=True, stop=True)
                                nc.tensor.matmul(sc[:, 512:1024], kT[psl, hp, ksl],
                                                 qT[psl, hp, 512:1024],
                                                 start=True, stop=True)
                                h = 2 * hp + j
                                if i == 0 and j == 0 and kcp % 8 < 5:
                                    # exp(s) ~= (1 + s/2)^2 on DVE (|s| <= 1/8)
                                    u = attp.tile([128, S_OWN], F32, name="expu",
                                                  tag="expu")
                                    nc.vector.tensor_scalar(
                                        out=u[:, :], in0=sc[:, :],
                                        scalar1=invk_half[h][:, kc:kc + 1],
                                        scalar2=1.0,
                                        op0=ALU.mult, op1=ALU.add)
                                    nc.vector.tensor_mul(out=e2[j][:, i, :],
                                                         in0=u[:, :], in1=u[:, :])
                                else:
                                    nc.scalar.activation(
                                        out=e2[j][:, i, :], in_=sc[:, :],
                                        func=AF.Exp,
                                        scale=invk_col[h][:, kc:kc + 1])
                        for j in range(2):
                            nc.tensor.matmul(
                                ctx2[j][:, 0:512],
                                vsb[:, kcp, :, 2 * hp + j, 0:DH + 1],
                                e2[j][:, :, 0:512],
                                start=(kcp == 0), stop=(kcp == 15),
                                perf_mode=mybir.MatmulPerfMode.DoubleRow)
                            nc.tensor.matmul(
                                ctx2[j][:, 512:1024],
                                vsb[:, kcp, :, 2 * hp + j, 0:DH + 1],
                                e2[j][:, :, 512:1024],
                                start=(kcp == 0), stop=(kcp == 15),
                                perf_mode=mybir.MatmulPerfMode.DoubleRow)
                    for j in range(2):
                        h = 2 * hp + j
                        psl = slice(64 * j, 64 * (j + 1))
                        rs = attrp.tile([DH + 1, S_OWN], F32, tag="rs")
                        nc.vector.reciprocal(out=rs[DH:DH + 1, :],
                                             in_=ctx2[j][DH:DH + 1, :])
                        rs16 = attrp.tile([DH + 1, S_OWN], BF16, tag="rs16")
                        nc.vector.tensor_copy(out=rs16[DH:DH + 1, :],
                                              in_=rs[DH:DH + 1, :])
                        nc.gpsimd.dma_start(out=scr_rsum.ap()[h:h + 1, :],
                                            in_=rs16[DH:DH + 1, :])
                        rep = attrp.tile([64, S_OWN], BF16, tag="rsrep")
                        nc.gpsimd.dma_start(
                            out=rep[:, :],
                            in_=bcast_ap(scr_rsum, h * S_OWN, 64, S_OWN))
                        nc.vector.tensor_mul(out=ctxT[psl, hp, :],
                                             in0=ctx2[j][0:DH, :], in1=rep[:, :])

            # ================= P7: output proj + gate + residual =================
            with tc.tile_pool(name="ops", bufs=2, space="PSUM") as opsp, \
                 tc.tile_pool(name="fin", bufs=1) as finp, \
                 tc.tile_pool(name="fin3", bufs=3) as fin3:
                zs, projs = [], []
                mv_all = finp.tile([128, 2, 8], F32)
                for sb in range(8):
                    ssl = slice(sb * 128, (sb + 1) * 128)
                    ps_o = opsp.tile([128, D], F32, tag="pso", name="pso")
                    for c in range(4):
                        nc.tensor.matmul(ps_o[:, :], ctxT[:, c, ssl], wob[:, c, :],
                                         start=(c == 0), stop=False)
                    nc.tensor.matmul(ps_o[:, :], ones_row[:, :], bo_row[:, :],
                                     start=False, stop=True)
                    proj = finp.tile([128, D], BF16, tag=f"proj{sb}",
                                     name=f"proj{sb}")
                    nc.scalar.copy(out=proj[:, :], in_=ps_o[:, :])
                    projs.append(proj)

                    ps_z = opsp.tile([128, D], F32, tag="psz", name="psz")
                    for c in range(4):
                        nc.tensor.matmul(ps_z[:, :], ctxT[:, c, ssl], wgb[:, c, :],
                                         start=(c == 0), stop=False)
                    for c in range(4):
                        nc.tensor.matmul(ps_z[:, :], xT_own[:, c, ssl],
                                         wgb[:, 4 + c, :], start=False, stop=False)
                    nc.tensor.matmul(ps_z[:, :], ones_row[:, :], bg_row[:, :],
                                     start=False, stop=True)
                    z = finp.tile([128, D], BF16, tag=f"z{sb}", name=f"z{sb}")
                    nc.scalar.copy(out=z[:, :], in_=ps_z[:, :])
                    zs.append(z)
                    stats = fin3.tile([128, 6], F32, tag="st6", name="st6")
                    nc.vector.bn_stats(out=stats[:, :], in_=z[:, :])
                    nc.vector.bn_aggr(out=mv_all[:, :, sb], in_=stats[:, :])

                rstd_all, _, _ = rsqrt_dve(nc, finp, mv_all[:, 1, :], "g_rs",
                                           eps=LN_EPS)
                for sb in range(8):
                    ssl = slice(sb * 128, (sb + 1) * 128)
                    z, proj = zs[sb], projs[sb]
                    zn = fin3.tile([128, D], F32, tag="zn", name="zn")
                    nc.vector.tensor_scalar(out=zn[:, :], in0=z[:, :],
                                            scalar1=mv_all[:, 0:1, sb],
                                            scalar2=rstd_all[:, sb:sb + 1],
                                            op0=ALU.subtract, op1=ALU.mult)
                    zg = fin3.tile([128, D], F32, tag="zg", name="zg")
                    nc.vector.tensor_mul(out=zg[:, :], in0=zn[:, :], in1=gg_rep[:, :])
                    nc.vector.tensor_add(out=zg[:, :], in0=zg[:, :], in1=gb_rep[:, :])
                    gate = fin3.tile([128, D], F32, tag="gate", name="gate")
                    nc.scalar.activation(out=gate[:, :], in_=zg[:, :],
                                         func=AF.Sigmoid)
                    xblk = fin3.tile([128, D], F32, tag="xblk", name="xblk")
                    nc.sync.dma_start(out=xblk[:, :], in_=xf_own.ap()[ssl, :])
                    gp = fin3.tile([128, D], F32, tag="gp", name="gp")
                    nc.vector.tensor_mul(out=gp[:, :], in0=gate[:, :],
                                         in1=proj[:, :])
                    ob = fin3.tile([128, D], F32, tag="ob", name="ob")
                    nc.vector.tensor_add(out=ob[:, :], in0=gp[:, :], in1=xblk[:, :])
                    nc.sync.dma_start(out=out_d.ap()[ssl, :], in_=ob[:, :])

    nc.compile()
    return nc


_NC_CACHE = None


def _get_nc():
    global _NC_CACHE
    if _NC_CACHE is None:
        _NC_CACHE = build_nc()
    return _NC_CACHE


def make_in_maps(inputs):
    xg = np.ascontiguousarray(np.asarray(inputs["gene_embeds"], np.float32))
    xd = np.ascontiguousarray(np.asarray(inputs["drug_embeds"], np.float32))
    xg16 = xg.astype(ml_dtypes.bfloat16)
    xd16 = xd.astype(ml_dtypes.bfloat16)

    f32 = lambda k: np.ascontiguousarray(np.asarray(inputs[k], np.float32))

    gene_common = dict(
        x16_oth=xd16, wq=f32("wgq"), wk=f32("wdk"), wv=f32("wdv"), wo=f32("wo"),
        wg=f32("wgg"), bq=f32("bgq"), bk=f32("bdk"), bv=f32("bdv"), bo=f32("bo"),
        bg=f32("bgg"), g_own=f32("lng_g"), b_own=f32("lng_b"), g_oth=f32("lnd_g"),
        b_oth=f32("lnd_b"), gg=f32("gg_g"), gb=f32("gg_b"))
    drug_common = dict(
        x16_oth=xg16, wq=f32("wdq"), wk=f32("wgk"), wv=f32("wgv"), wo=f32("wo"),
        wg=f32("wdg"), bq=f32("bdq"), bk=f32("bgk"), bv=f32("bgv"), bo=f32("bo"),
        bg=f32("bdg"), g_own=f32("lnd_g"), b_own=f32("lnd_b"), g_oth=f32("lng_g"),
        b_oth=f32("lng_b"), gg=f32("dg_g"), gb=f32("dg_b"))

    in_maps = []
    for i in range(8):
        if i < 4:
            sl = slice(i * S_OWN, (i + 1) * S_OWN)
            m = dict(gene_common)
            m["x16_own"] = np.ascontiguousarray(xg16[sl])
            m["xf_own"] = np.ascontiguousarray(xg[sl])
        else:
            sl = slice((i - 4) * S_OWN, (i - 3) * S_OWN)
            m = dict(drug_common)
            m["x16_own"] = np.ascontiguousarray(xd16[sl])
            m["xf_own"] = np.ascontiguousarray(xd[sl])
        in_maps.append(m)
    return in_maps


def kernel(**inputs):
    nc = _get_nc()
    in_maps = make_in_maps(inputs)
    res = run_bass_kernel_spmd(nc, in_maps, core_ids=list(range(8)))
    gene_out = np.concatenate([res.results[i]["out"] for i in range(4)], axis=0)
    drug_out = np.concatenate([res.results[i]["out"] for i in range(4, 8)], axis=0)
    return (gene_out, drug_out)
